# revision 1
# baseline (speedup 1.0000x reference)
"""Trainium2 Bass kernel for nn_Autoencoder_65223373357102 (FLAME-style autoencoder).

Strategy:
  Phase 1 (8-way tensor parallel): encoder GEMM [64,150528]@[150528,556] sharded
  along the input-feature axis. Each core transposes its x shard on TensorE,
  multiplies against its 1/8 slice of enc_W, adds enc_b/8 via a K=1 matmul, and
  AllReduces the [64,556] latent (142 KB).
  Phase 2 (replicated): blendshape GEMM [64,400]@[400,3*5023] in plane-separated
  layout + all per-batch geometry with batch on partitions; per-batch scalars are
  broadcast along the free axis via tensor_scalar. Every core computes the full
  output; the host takes core 0's copy.
"""
import sys
import types

sys.path.insert(0, "/opt/trn_rl_repo")

import numpy as np


def _ensure_ntff_hook():
    """Provide antenv.axon_hooks + install the ctypes NTFF profile hook so
    run_bass_kernel_spmd(trace=True) can pull a neuron-profile under axon."""
    name = "antenv.axon_hooks"
    if name not in sys.modules:
        mod = types.ModuleType(name)
        mod._HOOK = None

        def set_axon_ntff_profile_hook(hook):
            mod._HOOK = hook

        def get_axon_ntff_profile_hook():
            return mod._HOOK

        mod.set_axon_ntff_profile_hook = set_axon_ntff_profile_hook
        mod.get_axon_ntff_profile_hook = get_axon_ntff_profile_hook
        sys.modules[name] = mod
        try:
            import antenv

            antenv.axon_hooks = mod
        except ImportError:
            pass
    mod = sys.modules[name]
    if mod.get_axon_ntff_profile_hook() is None:
        try:
            from trn_agent_boot.trn_boot import _ntff_profile_via_ctypes

            hook = _ntff_profile_via_ctypes("/opt/axon/libaxon_pjrt.so")
            if hook is not None:
                mod.set_axon_ntff_profile_hook(hook)
        except Exception:
            pass


_ensure_ntff_hook()

from concourse import bass, mybir, tile
from concourse.bass_utils import run_bass_kernel_spmd

F32 = mybir.dt.float32
ALU = mybir.AluOpType
ACTF = mybir.ActivationFunctionType
AX = mybir.AxisListType

B = 64
V = 5023
VM = 3500
LAT = 556
DIN = 3 * 224 * 224  # 150528
NCORES = 8
KSH = DIN // NCORES  # 18816
KTILES = KSH // 128  # 147
NOUT = 2 * VM + 68 + 11  # 7079
GAZE_DIR = -1.0
HALF_PI = 1.5707963267948966


def _chunks(total, step):
    out = []
    o = 0
    while o < total:
        out.append((o, min(step, total - o)))
        o += step
    return out


class Geo:
    """Helper for tiny per-batch scalar ops on [rows,1] tiles."""

    _uid = [0]

    def __init__(self, nc, pool, rows=B):
        self.nc = nc
        self.pool = pool
        self.rows = rows

    def t(self, cols=1):
        Geo._uid[0] += 1
        return self.pool.tile([self.rows, cols], F32, name=f"g{Geo._uid[0]}_{cols}")

    def mul(self, a, b):
        o = self.t()
        self.nc.vector.tensor_tensor(out=o, in0=a, in1=b, op=ALU.mult)
        return o

    def add(self, a, b):
        o = self.t()
        self.nc.vector.tensor_tensor(out=o, in0=a, in1=b, op=ALU.add)
        return o

    def sub(self, a, b):
        o = self.t()
        self.nc.vector.tensor_tensor(out=o, in0=a, in1=b, op=ALU.subtract)
        return o

    def mac(self, a, s, acc):
        """(a * s) + acc, s is a [B,1] AP scalar."""
        o = self.t()
        self.nc.vector.scalar_tensor_tensor(
            out=o, in0=a, scalar=s, in1=acc, op0=ALU.mult, op1=ALU.add
        )
        return o

    def dot3(self, ax, ay, az, bx, by, bz):
        o = self.mul(ax, bx)
        o = self.mac(ay, by, o)
        o = self.mac(az, bz, o)
        return o

    def cross3(self, ax, ay, az, bx, by, bz):
        """a x b -> 3 [B,1] tiles."""
        cx = self.sub(self.mul(ay, bz), self.mul(az, by))
        cy = self.sub(self.mul(az, bx), self.mul(ax, bz))
        cz = self.sub(self.mul(ax, by), self.mul(ay, bx))
        return cx, cy, cz


def axis_angle_R(nc, g, aa3, pfx, halfpi):
    R_ = g.rows
    """aa3: [B,3] axis-angle tile -> R [B,9] tile, R[l,i] at col l*3+i.

    R = c*I + s*K + (1-c) a a^T  (Rodrigues, matching reference)
    """
    pool = g.pool
    sq = pool.tile([R_, 3], F32, name=pfx + "aaR_sq")
    nc.vector.tensor_tensor(out=sq, in0=aa3, in1=aa3, op=ALU.mult)
    th2 = g.t()
    nc.vector.tensor_reduce(out=th2, in_=sq, axis=AX.X, op=ALU.add)
    theta = g.t()
    nc.scalar.activation(out=theta, in_=th2, func=ACTF.Sqrt)
    thm = g.t()
    nc.vector.tensor_scalar_max(out=thm, in0=theta, scalar1=1e-8)
    rth = g.t()
    nc.vector.reciprocal(out=rth, in_=thm)
    axis3 = pool.tile([R_, 3], F32, name=pfx + "aaR_axis")
    nc.vector.tensor_scalar_mul(out=axis3, in0=aa3, scalar1=rth)
    s = g.t()
    nc.scalar.activation(out=s, in_=theta, func=ACTF.Sin)
    c = g.t()
    nc.scalar.activation(out=c, in_=theta, func=ACTF.Sin, bias=halfpi)
    omc = g.t()
    nc.vector.tensor_scalar(
        out=omc, in0=c, scalar1=-1.0, scalar2=1.0, op0=ALU.mult, op1=ALU.add
    )
    ax, ay, az = axis3[:, 0:1], axis3[:, 1:2], axis3[:, 2:3]
    # diag: omc*a_i^2 + c
    asq = pool.tile([R_, 3], F32, name=pfx + "aaR_asq")
    nc.vector.tensor_tensor(out=asq, in0=axis3, in1=axis3, op=ALU.mult)
    R = pool.tile([R_, 9], F32, name=pfx + "aaR_R")
    dmul = pool.tile([R_, 3], F32, name=pfx + "aaR_dmul")
    nc.vector.tensor_scalar_mul(out=dmul, in0=asq, scalar1=omc)
    # s*a
    sa = pool.tile([R_, 3], F32, name=pfx + "aaR_sa")
    nc.vector.tensor_scalar_mul(out=sa, in0=axis3, scalar1=s)
    sax, say, saz = sa[:, 0:1], sa[:, 1:2], sa[:, 2:3]
    # off-diag products omc*ai*aj
    mxy = g.mul(g.mul(ax, ay), omc)
    mxz = g.mul(g.mul(ax, az), omc)
    myz = g.mul(g.mul(ay, az), omc)
    # assemble diag: R[l*4] = dmul_l + c
    for l in range(3):
        nc.vector.tensor_tensor(
            out=R[:, 4 * l:4 * l + 1], in0=dmul[:, l:l + 1], in1=c, op=ALU.add
        )
    nc.vector.tensor_tensor(out=R[:, 1:2], in0=mxy, in1=saz, op=ALU.subtract)  # R01
    nc.vector.tensor_tensor(out=R[:, 2:3], in0=mxz, in1=say, op=ALU.add)  # R02
    nc.vector.tensor_tensor(out=R[:, 3:4], in0=mxy, in1=saz, op=ALU.add)  # R10
    nc.vector.tensor_tensor(out=R[:, 5:6], in0=myz, in1=sax, op=ALU.subtract)  # R12
    nc.vector.tensor_tensor(out=R[:, 6:7], in0=mxz, in1=say, op=ALU.subtract)  # R20
    nc.vector.tensor_tensor(out=R[:, 7:8], in0=myz, in1=sax, op=ALU.add)  # R21
    return R


_ENG_ATTR = {
    "SP": "sync", "Pool": "gpsimd", "PE": "tensor",
    "DVE": "vector", "Activation": "scalar",
}


def _legalize_waits(nc):
    """This walrus accepts only one sync-wait slot per instruction; move extra
    waits onto same-engine NoOps inserted right before the instruction."""
    import concourse.mybir as _mybir

    def make_nop(engine):
        eng = getattr(nc, _ENG_ATTR[engine.name])
        bi = eng.nop(nofuse=True)
        mi = bi.ins
        for bb in nc.main_func.blocks:
            if bb.instructions and bb.instructions[-1].name == mi.name:
                bb.instructions.pop()
                break
        mi.engine = engine
        return mi

    for bb in nc.main_func.blocks:
        snapshot = list(bb.instructions)
        newlist = []
        changed = False
        for inst in snapshot:
            si = inst.sync_info
            waits = list(si.on_wait) if (si and si.on_wait) else []
            if (
                len(waits) > 1
                and not inst.name.startswith("barrier")
                and inst.engine is not None
                and getattr(inst.engine, "name", None) in _ENG_ATTR
            ):
                for w in waits[:-1]:
                    nop = make_nop(inst.engine)
                    nop.sync_info = _mybir.SyncInfo(on_wait=[w], on_update=[])
                    newlist.append(nop)
                inst.sync_info = _mybir.SyncInfo(
                    on_wait=[waits[-1]], on_update=list(si.on_update)
                )
                changed = True
            newlist.append(inst)
        if changed:
            bb.instructions[:] = newlist


def build_graph(fl_idx, idx4, idx2, l_lo, r_lo):
    """fl_idx: 68 ints (vert cols for masked landmarks), idx4/idx2: landmark vert
    cols, l_lo/r_lo: start of the contiguous eye ranges."""
    nc = bass.Bass(target_bir_lowering=False)

    x_p = nc.declare_dram_parameter("x_sh", [KSH, B], F32, isOutput=False)
    w_p = nc.declare_dram_parameter("w_sh", [KSH, LAT], F32, isOutput=False)
    b_p = nc.declare_dram_parameter("enc_b", [1, LAT + 128 + 3], F32, isOutput=False)
    bm_p = nc.declare_dram_parameter("bmean", [128, 12], F32, isOutput=False)
    tpl_p = nc.declare_dram_parameter("tmpl", [3, V], F32, isOutput=False)
    bas_p = nc.declare_dram_parameter("basis", [400, 3, V], F32, isOutput=False)
    cam_p = nc.declare_dram_parameter("cam", [B, 12], F32, isOutput=False)
    out_p = nc.declare_dram_parameter("out", [B, 3, NOUT], F32, isOutput=True)

    ar_in = nc.dram_tensor("ar_in", [B, LAT], F32)
    ar_out = nc.dram_tensor("ar_out", [B, LAT], F32, addr_space="Shared")

    with tile.TileContext(nc) as tc:
        with (
            tc.tile_pool(name="consts", bufs=1) as consts,
            tc.tile_pool(name="latents", bufs=1) as latp,
            tc.tile_pool(name="geo", bufs=1) as geop,
            tc.tile_pool(name="planes", bufs=1) as planep,
            tc.tile_pool(name="dum", bufs=1, space="PSUM") as dum,
        ):
            b_sb = consts.tile([1, LAT + 128 + 3], F32)
            nc.sync.dma_start(out=b_sb, in_=b_p[:, :])
            ones8 = b_sb[:, LAT:LAT + B]       # value 1/NCORES, packed by host
            ones1 = b_sb[:, LAT + B:LAT + 2 * B]  # value 1.0, packed by host
            halfpi = consts.tile([128, 1], F32)
            nc.vector.memset(halfpi, HALF_PI)
            # PE matmuls carry a single sync-wait slot on this walrus; dummy
            # 1-wait matmuls make PE observe one dep before the real matmul.
            d1 = dum.tile([1, 1], F32)
            d64 = dum.tile([B, 1], F32)

            # ---------------- Phase 1: encoder GEMM ----------------
            NSPL = [(0, 512), (512, 44)]
            TPC = 7  # k-tiles per x chunk
            with (
                tc.tile_pool(name="xin", bufs=3) as xin,
                tc.tile_pool(name="wts", bufs=3) as wts,
                tc.tile_pool(name="encp", bufs=1, space="PSUM") as encp,
            ):
                pe = [encp.tile([B, n], F32, name=f"pe{j}", tag=f"pe{j}") for j, (_, n) in enumerate(NSPL)]
                x_view = x_p.ap().rearrange("(c t p) m -> c p t m", t=TPC, p=128)
                w_view = w_p.ap().rearrange("(c t p) m -> c p t m", t=TPC, p=128)
                for ci in range(KTILES // TPC):
                    x_c = xin.tile([128, TPC, B], F32)
                    nc.gpsimd.dma_start(out=x_c, in_=x_view[ci])
                    nc.tensor.matmul(
                        d1, lhsT=x_c[:, 0, 0:1], rhs=x_c[:, 0, 0:1],
                        start=True, stop=True, skip_group_check=True,
                    )
                    w_c = wts.tile([128, TPC, LAT], F32)
                    nc.sync.dma_start(out=w_c, in_=w_view[ci])
                    for t in range(TPC):
                        k = ci * TPC + t
                        for j, (n0, n) in enumerate(NSPL):
                            nc.tensor.matmul(
                                pe[j],
                                lhsT=x_c[:, t, :],
                                rhs=w_c[:, t, n0:n0 + n],
                                start=(k == 0),
                                stop=False,
                            )
                for j, (n0, n) in enumerate(NSPL):
                    nc.tensor.matmul(
                        pe[j],
                        lhsT=ones8,
                        rhs=b_sb[:, n0:n0 + n],
                        start=False,
                        stop=True,
                    )
                lat1 = latp.tile([B, LAT], F32)
                for j, (n0, n) in enumerate(NSPL):
                    nc.vector.tensor_copy(out=lat1[:, n0:n0 + n], in_=pe[j])
                nc.sync.dma_start(out=ar_in[:, :], in_=lat1)

            # prefetch the first basis chunks before the collective so the
            # DMA engines stay busy through the AllReduce bubble
            basp_ctx = tc.tile_pool(name="bas", bufs=12)
            basp = basp_ctx.__enter__()
            KSPL = [(0, 128, 128), (128, 128, 128), (256, 128, 128), (384, 16, 32)]
            VCH = _chunks(V, 512)
            bts = {}
            for j in (0, 1, 2):
                n0, n = VCH[j]
                for ki, (k0, kw, _cwa) in enumerate(KSPL):
                    bt = basp.tile([128, 3, 512], F32, name=f"btp{j}_{ki}", tag="bt")
                    nc.gpsimd.dma_start(
                        out=bt[:kw, :, :n], in_=bas_p[k0:k0 + kw, :, n0:n0 + n]
                    )
                    bts[(j, ki)] = bt
            nc.gpsimd.collective_compute(
                "AllReduce",
                ALU.add,
                replica_groups=[list(range(NCORES))],
                ins=[ar_in.ap().opt()],
                outs=[ar_out.ap().opt()],
            )
            lat = latp.tile([B, LAT], F32)
            nc.sync.dma_start(out=lat, in_=ar_out[:, :])

            # ---------------- Phase 1.5: transpose shape params ----------------
            # DVE 32x32 block transposes: spT[ki][r, b] = lat[b, c0+r].
            spT = []
            for (c0, kw, cwa) in KSPL:
                st = latp.tile([cwa, B], F32, name=f"spT{c0}", tag=f"spT{c0}")
                for pb in range(cwa // 32):
                    for fb in range(B // 32):
                        nc.vector.transpose(
                            out=st[32 * pb:32 * pb + 32, 32 * fb:32 * fb + 32],
                            in_=lat[32 * fb:32 * fb + 32,
                                    c0 + 32 * pb:c0 + 32 * pb + 32],
                        )
                spT.append(st)
            nc.tensor.matmul(
                d64, lhsT=spT[3], rhs=spT[3][:, 0:1],
                start=True, stop=True, skip_group_check=True,
            )

            # ---------------- Phase 2: blendshape + fused face transform ----------
            g = Geo(nc, geop)
            # vmean directly from latent: vm = tmpl_mean + shape_p @ basis_mean
            bm_sb = consts.tile([128, 12], F32)
            nc.sync.dma_start(out=bm_sb, in_=bm_p[:, :])
            with tc.tile_pool(name="vmp", bufs=1, space="PSUM") as vmp:
                pvm = vmp.tile([B, 3], F32)
                for ki, (k0, kw, _cwa) in enumerate(KSPL):
                    nc.tensor.matmul(
                        pvm, lhsT=spT[ki][:kw, :], rhs=bm_sb[:kw, ki * 3:ki * 3 + 3],
                        start=(ki == 0), stop=False,
                    )
                nc.tensor.matmul(
                    pvm, lhsT=ones1, rhs=b_sb[:, LAT + 128:LAT + 131],
                    start=False, stop=True,
                )
                vms = geop.tile([B, 3], F32)
                nc.vector.tensor_copy(out=vms, in_=pvm)

            # face rotation matrix, scaled
            aa_face = lat[:, 545:548]
            Rf = axis_angle_R(nc, g, aa_face, "f_", halfpi[:B, :])
            fs = g.t()  # face_scale = latent[551]+1
            nc.vector.tensor_scalar_add(out=fs, in0=lat[:, 551:552], scalar1=1.0)
            Rs = geop.tile([B, 9], F32)
            nc.vector.tensor_scalar_mul(out=Rs, in0=Rf, scalar1=fs)
            # offsets: off_i = face_t_i - sum_l vms_l*Rs[l,i]
            off = geop.tile([B, 3], F32)
            for i in range(3):
                t = g.mul(vms[:, 0:1], Rs[:, i:i + 1])
                t = g.mac(vms[:, 1:2], Rs[:, 3 + i:4 + i], t)
                t = g.mac(vms[:, 2:3], Rs[:, 6 + i:7 + i], t)
                nc.vector.tensor_tensor(
                    out=off[:, i:i + 1], in0=lat[:, 548 + i:549 + i], in1=t,
                    op=ALU.subtract,
                )

            # blendshape chunks; rotation fused per chunk into rt
            rt = planep.tile([B, 3, V], F32)
            with (
                tc.tile_pool(name="tpl", bufs=2) as tplp,
                tc.tile_pool(name="vstage", bufs=3) as vstp,
                tc.tile_pool(name="bpsum", bufs=3, space="PSUM") as bpsum,
            ):

                prev = []  # vstage read-APs for WAR-absorbing dummies
                for j, (n0, n) in enumerate(VCH):
                    vs = vstp.tile([B, 3, 512], F32)
                    if (j, 0) not in bts:
                        for ki, (k0, kw, _cwa) in enumerate(KSPL):
                            bt = basp.tile([128, 3, 512], F32, name=f"btl{j}_{ki}", tag="bt")
                            nc.gpsimd.dma_start(
                                out=bt[:kw, :, :n], in_=bas_p[k0:k0 + kw, :, n0:n0 + n]
                            )
                            bts[(j, ki)] = bt
                    for p in range(3):
                        gi = j * 3 + p
                        if gi >= 3:
                            pap = prev[gi - 3]
                            nc.tensor.matmul(
                                d1, lhsT=pap, rhs=pap,
                                start=True, stop=True, skip_group_check=True,
                            )
                        pv = bpsum.tile([B, 512], F32)
                        for ki, (k0, kw, _cwa) in enumerate(KSPL):
                            nc.tensor.matmul(
                                pv[:, :n],
                                lhsT=spT[ki][:kw, :],
                                rhs=bts[(j, ki)][:kw, p, :n],
                                start=(ki == 0),
                                stop=False,
                            )
                        tl = tplp.tile([1, 512], F32)
                        nc.sync.dma_start(out=tl[:, :n], in_=tpl_p[p:p + 1, n0:n0 + n])
                        nc.tensor.matmul(
                            pv[:, :n], lhsT=ones1, rhs=tl[:, :n],
                            start=False, stop=True,
                        )
                        nc.scalar.copy(out=vs[:, p, :n], in_=pv[:, :n])
                        prev.append(vs[:, p, 0:1])
                    for i in range(3):
                        nc.vector.tensor_scalar(
                            out=rt[:, i, n0:n0 + n], in0=vs[:, 0, :n],
                            scalar1=Rs[:, i:i + 1], scalar2=off[:, i:i + 1],
                            op0=ALU.mult, op1=ALU.add,
                        )
                        for l in (1, 2):
                            nc.vector.scalar_tensor_tensor(
                                out=rt[:, i, n0:n0 + n], in0=vs[:, l, :n],
                                scalar=Rs[:, 3 * l + i:3 * l + i + 1],
                                in1=rt[:, i, n0:n0 + n],
                                op0=ALU.mult, op1=ALU.add,
                            )
            basp_ctx.__exit__(None, None, None)

            # eye processing: both eyes stacked on 128 partitions
            # (rows 0:64 = left batch, 64:128 = right batch)
            EW = 546
            g2 = Geo(nc, geop, rows=128)
            es = geop.tile([128, 3, EW], F32)
            for i in range(3):
                nc.vector.tensor_copy(out=es[0:B, i, :], in_=rt[:, i, l_lo:l_lo + EW])
            nc.sync.dma_start(out=es[B:128, :, :], in_=rt[:, :, r_lo:r_lo + EW])
            # centers (mean over eye verts), both eyes at once
            cc = geop.tile([128, 3], F32)
            for i in range(3):
                nc.vector.tensor_reduce(
                    out=cc[:, i:i + 1], in_=es[:, i, :], axis=AX.X, op=ALU.add
                )
            c3 = geop.tile([128, 3], F32)
            nc.vector.tensor_scalar_mul(out=c3, in0=cc, scalar1=1.0 / EW)
            # pivot verts (l:4051, r:4597)
            pvt = geop.tile([128, 3, 1], F32)
            for i in range(3):
                nc.vector.tensor_copy(out=pvt[0:B, i, :], in_=rt[:, i, 4051:4052])
            nc.sync.dma_start(out=pvt[B:128, :, :], in_=rt[:, :, 4597:4598])
            # a = normalize(pivot - centre)
            a3 = geop.tile([128, 3], F32)
            for i in range(3):
                nc.vector.tensor_tensor(
                    out=a3[:, i:i + 1], in0=pvt[:, i, 0:1], in1=c3[:, i:i + 1],
                    op=ALU.subtract,
                )
            sqe = geop.tile([128, 3], F32)
            nc.vector.tensor_tensor(out=sqe, in0=a3, in1=a3, op=ALU.mult)
            n2 = g2.t()
            nc.vector.tensor_reduce(out=n2, in_=sqe, axis=AX.X, op=ALU.add)
            nn = g2.t()
            nc.scalar.activation(out=nn, in_=n2, func=ACTF.Sqrt)
            rn = g2.t()
            nc.vector.reciprocal(out=rn, in_=nn)
            nc.vector.tensor_scalar_mul(out=a3, in0=a3, scalar1=rn)
            ax, ay, az = a3[:, 0:1], a3[:, 1:2], a3[:, 2:3]
            # find_gaze_R: b=(0,0,GAZE_DIR); v = a x b = (ay*g, -ax*g, 0)
            vx = g2.t()
            nc.vector.tensor_scalar_mul(out=vx, in0=ay, scalar1=GAZE_DIR)
            vy = g2.t()
            nc.vector.tensor_scalar_mul(out=vy, in0=ax, scalar1=-GAZE_DIR)
            cdot = g2.t()
            nc.vector.tensor_scalar_mul(out=cdot, in0=az, scalar1=GAZE_DIR)
            fden = g2.t()
            nc.vector.tensor_scalar_add(out=fden, in0=cdot, scalar1=1.0 + 1e-8)
            f = g2.t()
            nc.vector.reciprocal(out=f, in_=fden)
            vv = g2.mac(vy, vy, g2.mul(vx, vx))
            fvv = g2.mul(f, vv)
            dd = g2.t()  # 1 - f*vv
            nc.vector.tensor_scalar(
                out=dd, in0=fvv, scalar1=-1.0, scalar2=1.0, op0=ALU.mult, op1=ALU.add
            )
            fxy = g2.mul(g2.mul(vx, vy), f)
            Rl = geop.tile([128, 9], F32)
            nc.vector.tensor_tensor(
                out=Rl[:, 0:1], in0=dd, in1=g2.mul(f, g2.mul(vx, vx)), op=ALU.add
            )
            nc.vector.tensor_tensor(
                out=Rl[:, 4:5], in0=dd, in1=g2.mul(f, g2.mul(vy, vy)), op=ALU.add
            )
            nc.vector.tensor_copy(out=Rl[:, 8:9], in_=dd)
            nc.vector.tensor_copy(out=Rl[:, 1:2], in_=fxy)
            nc.vector.tensor_copy(out=Rl[:, 3:4], in_=fxy)
            nc.vector.tensor_copy(out=Rl[:, 2:3], in_=vy)
            nc.vector.tensor_scalar_mul(out=Rl[:, 5:6], in0=vx, scalar1=-1.0)
            nc.vector.tensor_scalar_mul(out=Rl[:, 6:7], in0=vy, scalar1=-1.0)
            nc.vector.tensor_copy(out=Rl[:, 7:8], in_=vx)
            # eyeball rotation from latent rot2 (az=0), stacked l/r
            aa2 = geop.tile([128, 3], F32)
            nc.vector.memset(aa2, 0.0)
            nc.vector.tensor_copy(out=aa2[0:B, 0:2], in_=lat[:, 552:554])
            nc.sync.dma_start(out=aa2[B:128, 0:2], in_=lat[:, 554:556])
            R2 = axis_angle_R(nc, g2, aa2, "e_", halfpi)
            # gaze = GAZE_DIR * R2[2,:]
            gz = geop.tile([128, 3], F32)
            nc.vector.tensor_scalar_mul(out=gz, in0=R2[:, 6:9], scalar1=GAZE_DIR)
            # M = Rl @ R2
            M = geop.tile([128, 9], F32)
            for l in range(3):
                for i in range(3):
                    t = g2.mul(Rl[:, 3 * l:3 * l + 1], R2[:, i:i + 1])
                    t = g2.mac(R2[:, 3 + i:4 + i], Rl[:, 3 * l + 1:3 * l + 2], t)
                    t = g2.mac(R2[:, 6 + i:7 + i], Rl[:, 3 * l + 2:3 * l + 3], t)
                    nc.vector.tensor_copy(out=M[:, 3 * l + i:3 * l + i + 1], in_=t)
            # offe_i = c_i - sum_l c_l M[l,i]
            offe = geop.tile([128, 3], F32)
            for i in range(3):
                t = g2.mul(c3[:, 0:1], M[:, i:i + 1])
                t = g2.mac(c3[:, 1:2], M[:, 3 + i:4 + i], t)
                t = g2.mac(c3[:, 2:3], M[:, 6 + i:7 + i], t)
                nc.vector.tensor_tensor(
                    out=offe[:, i:i + 1], in0=c3[:, i:i + 1], in1=t, op=ALU.subtract
                )
            # apply to both eye slices
            es2 = geop.tile([128, 3, EW], F32)
            for i in range(3):
                nc.vector.tensor_scalar(
                    out=es2[:, i, :], in0=es[:, 0, :],
                    scalar1=M[:, i:i + 1], scalar2=offe[:, i:i + 1],
                    op0=ALU.mult, op1=ALU.add,
                )
                for l in (1, 2):
                    nc.vector.scalar_tensor_tensor(
                        out=es2[:, i, :], in0=es[:, l, :],
                        scalar=M[:, 3 * l + i:3 * l + i + 1], in1=es2[:, i, :],
                        op0=ALU.mult, op1=ALU.add,
                    )
            for i in range(3):
                nc.vector.tensor_copy(out=rt[:, i, l_lo:l_lo + EW], in_=es2[0:B, i, :])
            nc.sync.dma_start(out=rt[:, :, r_lo:r_lo + EW], in_=es2[B:128, :, :])
            # unpack right-eye centre/gaze down to rows 0:64 for the solve
            rc64 = geop.tile([B, 3], F32)
            nc.sync.dma_start(out=rc64, in_=c3[B:128, :])
            rg64 = geop.tile([B, 3], F32)
            nc.sync.dma_start(out=rg64, in_=gz[B:128, :])
            lc = c3[0:B, :]
            lg = gz[0:B, :]
            rc = rc64
            rg = rg64

            # face centre from landmarks
            fc = geop.tile([B, 3], F32)
            for i in range(3):
                t4 = g.add(rt[:, i, idx4[0]:idx4[0] + 1], rt[:, i, idx4[1]:idx4[1] + 1])
                t4 = g.add(t4, rt[:, i, idx4[2]:idx4[2] + 1])
                t4 = g.add(t4, rt[:, i, idx4[3]:idx4[3] + 1])
                t2 = g.add(rt[:, i, idx2[0]:idx2[0] + 1], rt[:, i, idx2[1]:idx2[1] + 1])
                # fc = t4/4/2 + t2/2/2
                o = g.t()
                nc.vector.tensor_scalar_mul(out=o, in0=t4, scalar1=0.125)
                nc.vector.scalar_tensor_tensor(
                    out=fc[:, i:i + 1], in0=t2, scalar=0.25, in1=o,
                    op0=ALU.mult, op1=ALU.add,
                )

            # gaze intersection (Cramer)
            d = [g.sub(rc[:, i:i + 1], lc[:, i:i + 1]) for i in range(3)]
            c0 = [lg[:, i:i + 1] for i in range(3)]
            c1 = []
            for i in range(3):
                o = g.t()
                nc.vector.tensor_scalar_mul(out=o, in0=rg[:, i:i + 1], scalar1=-1.0)
                c1.append(o)
            # c2 = rg x lg
            c2 = list(g.cross3(rg[:, 0:1], rg[:, 1:2], rg[:, 2:3],
                               lg[:, 0:1], lg[:, 1:2], lg[:, 2:3]))
            # w = c1 x c2 ; det = c0.w ; num0 = d.w
            w = g.cross3(*c1, *c2)
            det = g.dot3(*c0, *w)
            num0 = g.dot3(*d, *w)
            # w2 = d x c2 ; num1 = c0.w2  (det with col1 replaced by d)
            w2 = g.cross3(*d, *c2)
            num1 = g.dot3(*c0, *w2)
            rdet = g.t()
            nc.vector.reciprocal(out=rdet, in_=det)
            sol0 = g.mul(num0, rdet)
            sol1 = g.mul(num1, rdet)
            # gp_l = l_c + sol0*lg ; gp_r = r_c + sol1*rg ; gp_mid
            gpl = geop.tile([B, 3], F32)
            gpr = geop.tile([B, 3], F32)
            gpm = geop.tile([B, 3], F32)
            for i in range(3):
                nc.vector.scalar_tensor_tensor(
                    out=gpl[:, i:i + 1], in0=lg[:, i:i + 1], scalar=sol0,
                    in1=lc[:, i:i + 1], op0=ALU.mult, op1=ALU.add,
                )
                nc.vector.scalar_tensor_tensor(
                    out=gpr[:, i:i + 1], in0=rg[:, i:i + 1], scalar=sol1,
                    in1=rc[:, i:i + 1], op0=ALU.mult, op1=ALU.add,
                )
            nc.vector.tensor_tensor(out=gpm, in0=gpl, in1=gpr, op=ALU.add)
            nc.vector.tensor_scalar_mul(out=gpm, in0=gpm, scalar1=0.5)
            dff = geop.tile([B, 3], F32)
            nc.vector.tensor_tensor(out=dff, in0=gpl, in1=gpr, op=ALU.subtract)
            nc.vector.tensor_tensor(out=dff, in0=dff, in1=dff, op=ALU.mult)
            d2 = g.t()
            nc.vector.tensor_reduce(out=d2, in_=dff, axis=AX.X, op=ALU.add)
            dist = g.t()
            nc.scalar.activation(out=dist, in_=d2, func=ACTF.Sqrt)
            # far points l_c + 1000*lg
            farl = geop.tile([B, 3], F32)
            farr = geop.tile([B, 3], F32)
            for i in range(3):
                nc.vector.scalar_tensor_tensor(
                    out=farl[:, i:i + 1], in0=lg[:, i:i + 1], scalar=1000.0,
                    in1=lc[:, i:i + 1], op0=ALU.mult, op1=ALU.add,
                )
                nc.vector.scalar_tensor_tensor(
                    out=farr[:, i:i + 1], in0=rg[:, i:i + 1], scalar=1000.0,
                    in1=rc[:, i:i + 1], op0=ALU.mult, op1=ALU.add,
                )

            # projection of face verts
            cam = geop.tile([B, 12], F32)
            nc.sync.dma_start(out=cam, in_=cam_p[:, :])
            with tc.tile_pool(name="imgp", bufs=1) as imgp:
                img = imgp.tile([B, 3, VM], F32)
                for i in (2, 0, 1):  # z first (feeds the clamp chain on DVE)
                    eng = nc.vector
                    eng.tensor_scalar(
                        out=img[:, i, :], in0=rt[:, 0, 0:VM],
                        scalar1=cam[:, 4 * i:4 * i + 1], scalar2=cam[:, 4 * i + 3:4 * i + 4],
                        op0=ALU.mult, op1=ALU.add,
                    )
                    for l in (1, 2):
                        eng.scalar_tensor_tensor(
                            out=img[:, i, :], in0=rt[:, l, 0:VM],
                            scalar=cam[:, 4 * i + l:4 * i + l + 1], in1=img[:, i, :],
                            op0=ALU.mult, op1=ALU.add,
                        )
                with tc.tile_pool(name="ztmp", bufs=1) as ztp:
                    az_ = ztp.tile([B, VM], F32)
                    nc.scalar.activation(out=az_, in_=img[:, 2, :], func=ACTF.Abs)
                    nc.vector.tensor_scalar_max(out=az_, in0=az_, scalar1=1e-3)
                    sg = ztp.tile([B, VM], F32)
                    nc.vector.tensor_scalar(
                        out=sg, in0=img[:, 2, :], scalar1=0.0, scalar2=None, op0=ALU.is_ge
                    )
                    nc.vector.tensor_scalar(
                        out=sg, in0=sg, scalar1=2.0, scalar2=1.0,
                        op0=ALU.mult, op1=ALU.subtract,
                    )
                    nc.vector.tensor_tensor(out=sg, in0=sg, in1=az_, op=ALU.mult)
                    nc.vector.reciprocal(out=az_, in_=sg)
                    nc.vector.tensor_tensor(
                        out=img[:, 0, :], in0=img[:, 0, :], in1=az_, op=ALU.mult
                    )
                    nc.vector.tensor_tensor(
                        out=img[:, 1, :], in0=img[:, 1, :], in1=az_, op=ALU.mult
                    )

                # landmark gather + tail assembly
                fl = geop.tile([B, 3, 68], F32)
                def _cp(k, out, in_):
                    e = k % 3
                    if e == 0:
                        nc.vector.tensor_copy(out=out, in_=in_)
                    elif e == 1:
                        nc.scalar.copy(out=out, in_=in_)
                    else:
                        nc.gpsimd.tensor_copy(out=out, in_=in_)

                for j, idx in enumerate(fl_idx):
                    for i in range(3):
                        _cp(j * 3 + i, fl[:, i, j:j + 1], rt[:, i, idx:idx + 1])
                tail = geop.tile([B, 3, 11], F32)
                for i in range(3):
                    pieces = [
                        lc[:, i:i + 1], rc[:, i:i + 1], fc[:, i:i + 1],
                        gpl[:, i:i + 1], gpr[:, i:i + 1], gpm[:, i:i + 1],
                        farl[:, i:i + 1], farr[:, i:i + 1],
                        lg[:, i:i + 1], rg[:, i:i + 1], dist,
                    ]
                    for j, src in enumerate(pieces):
                        _cp(i * 11 + j, tail[:, i, j:j + 1], src)

                # output DMAs
                for i in range(3):
                    nc.sync.dma_start(out=out_p[:, i, 0:VM], in_=rt[:, i, 0:VM])
                    nc.sync.dma_start(out=out_p[:, i, VM:2 * VM], in_=img[:, i, :])
                    nc.sync.dma_start(
                        out=out_p[:, i, 2 * VM:2 * VM + 68], in_=fl[:, i, :]
                    )
                    nc.sync.dma_start(
                        out=out_p[:, i, 2 * VM + 68:NOUT], in_=tail[:, i, :]
                    )
    _legalize_waits(nc)
    return nc


def _prep(inputs):
    x = np.ascontiguousarray(inputs["x"].reshape(B, DIN), dtype=np.float32)
    enc_W = np.asarray(inputs["enc_W"], dtype=np.float32)
    basis_np = np.asarray(inputs["shape_basis"], dtype=np.float32)
    tmpl_np = np.asarray(inputs["v_template"], dtype=np.float32)
    enc_b = np.concatenate([
        np.asarray(inputs["enc_b"], dtype=np.float32).reshape(1, LAT),
        np.full((1, B), 1.0 / NCORES, np.float32),
        np.ones((1, B), np.float32),
        tmpl_np.mean(axis=0).reshape(1, 3),
    ], axis=1)
    bmean_full = basis_np.mean(axis=1)  # [400, 3]
    bmean = np.zeros((128, 12), np.float32)
    for ki, (k0, kw) in enumerate([(0, 128), (128, 128), (256, 128), (384, 16)]):
        bmean[:kw, ki * 3:ki * 3 + 3] = bmean_full[k0:k0 + kw]
    tmpl = np.ascontiguousarray(
        np.asarray(inputs["v_template"], dtype=np.float32).T
    )  # [3, V]
    basis = np.ascontiguousarray(
        np.asarray(inputs["shape_basis"], dtype=np.float32).transpose(0, 2, 1)
    )  # [400, 3, V]
    cam = np.ascontiguousarray(
        np.asarray(inputs["camera_parameters"], dtype=np.float32).reshape(B, 12)
    )
    lm = np.asarray(inputs["landmarks"])
    mlm = np.asarray(inputs["masked_landmarks"])
    fmask = np.asarray(inputs["face_mask"])
    lmask = np.asarray(inputs["left_eyeball_mask"])
    rmask = np.asarray(inputs["right_eyeball_mask"])
    assert np.array_equal(lmask, np.arange(lmask[0], lmask[0] + 546)), "lmask not contiguous"
    assert np.array_equal(rmask, np.arange(rmask[0], rmask[0] + 546)), "rmask not contiguous"
    fl_idx = [int(fmask[i]) for i in mlm]
    idx4 = [int(lm[j]) for j in (19, 22, 25, 28)]
    idx2 = [int(lm[j]) for j in (14, 18)]
    return (x, enc_W, enc_b, bmean, tmpl, basis, cam, fl_idx, idx4, idx2,
            int(lmask[0]), int(rmask[0]))


def _run(inputs, trace=False):
    (x, enc_W, enc_b, bmean, tmpl, basis, cam, fl_idx, idx4, idx2, l_lo, r_lo) = _prep(inputs)
    nc = build_graph(fl_idx, idx4, idx2, l_lo, r_lo)
    in_maps = []
    for c in range(NCORES):
        k0 = c * KSH
        in_maps.append({
            "x_sh": np.ascontiguousarray(x[:, k0:k0 + KSH].T),
            "w_sh": np.ascontiguousarray(enc_W[k0:k0 + KSH, :]),
            "enc_b": enc_b,
            "bmean": bmean,
            "tmpl": tmpl,
            "basis": basis,
            "cam": cam,
        })
    res = run_bass_kernel_spmd(
        nc, in_maps, core_ids=list(range(NCORES)), trace=trace
    )
    out = res.results[0]["out"]  # [B, 3, NOUT]
    return np.ascontiguousarray(out.transpose(0, 2, 1)), res


def kernel(**inputs):
    out, _ = _run(inputs, trace=False)
    return out



# revision 7
# speedup vs baseline: 1.5420x; 1.5420x over previous
"""Trainium2 Bass kernel for nn_Autoencoder_65223373357102 (FLAME-style autoencoder).

Strategy (v2):
  Phase 1 (8-way tensor parallel): encoder GEMM [64,150528]@[150528,411] in fp32,
  sharded along the input-feature axis. Only the 411 *used* latent columns are
  computed (shape_p 0:400 + geometry latents 545:556; cols 400:545 are never read
  by the reference). Each core multiplies its x/W shard, adds bias via a K=1
  matmul, and AllReduces the [64,411] fp32 latent (105 KB).
  Phase 2 (vertex-sharded): the rotated eye vertices are dead code (face_mask
  0:3500 is disjoint from the eyeball masks), and eye centres / face centre /
  landmarks are affine images of precomputed basis mean-columns. So each core
  runs an fp32 blendshape GEMM over [72 geometry columns | its 448-column shard of
  the 3500 face verts], applies the fused face rotation, projects its shard, and
  runs the tiny per-batch gaze/Cramer chain. The host concatenates the 8 shard
  outputs; geometry/landmark sections are replicated and taken from core 0.
"""
import sys
import types

sys.path.insert(0, "/opt/trn_rl_repo")

import numpy as np


def _ensure_ntff_hook():
    """Provide antenv.axon_hooks + install the ctypes NTFF profile hook so
    run_bass_kernel_spmd(trace=True) can pull a neuron-profile under axon."""
    name = "antenv.axon_hooks"
    if name not in sys.modules:
        mod = types.ModuleType(name)
        mod._HOOK = None

        def set_axon_ntff_profile_hook(hook):
            mod._HOOK = hook

        def get_axon_ntff_profile_hook():
            return mod._HOOK

        mod.set_axon_ntff_profile_hook = set_axon_ntff_profile_hook
        mod.get_axon_ntff_profile_hook = get_axon_ntff_profile_hook
        sys.modules[name] = mod
        try:
            import antenv

            antenv.axon_hooks = mod
        except ImportError:
            pass
    mod = sys.modules[name]
    if mod.get_axon_ntff_profile_hook() is None:
        try:
            from trn_agent_boot.trn_boot import _ntff_profile_via_ctypes

            hook = _ntff_profile_via_ctypes("/opt/axon/libaxon_pjrt.so")
            if hook is not None:
                mod.set_axon_ntff_profile_hook(hook)
        except Exception:
            pass


_ensure_ntff_hook()

from concourse import bass, mybir, tile
from concourse.bass_utils import run_bass_kernel_spmd

F32 = mybir.dt.float32
ALU = mybir.AluOpType
ACTF = mybir.ActivationFunctionType
AX = mybir.AxisListType

B = 64
V = 5023
VM = 3500
LAT = 556
NLAT = 411          # used latent cols: 0:400 (shape) + 545:556 (geometry)
DIN = 3 * 224 * 224  # 150528
NCORES = 8
KSH = DIN // NCORES  # 18816
TPC = 7              # k-tiles per chunk
NCHUNK = KSH // (128 * TPC)  # 21
NOUT = 2 * VM + 68 + 11  # 7079
GAZE_DIR = -1.0
HALF_PI = 1.5707963267948966

NG = 72              # geometry columns: 68 landmarks | l_mean | r_mean | fc | vmean
VSH = 448            # face-vert shard columns per core (8*448 = 3584 >= 3500)
NCOL = NG + VSH      # 520
KSPL = [(0, 128), (128, 128), (256, 128), (384, 32)]  # shape-K tiles (400 pad 416)
OUTW = VSH + VSH + 68 + 11  # 975 per-core output cols


class Geo:
    """Helper for tiny per-batch scalar ops on [rows,1] tiles."""

    _uid = [0]

    def __init__(self, nc, pool, rows=B):
        self.nc = nc
        self.pool = pool
        self.rows = rows

    def t(self, cols=1):
        Geo._uid[0] += 1
        return self.pool.tile([self.rows, cols], F32, name=f"g{Geo._uid[0]}_{cols}")

    def mul(self, a, b):
        o = self.t()
        self.nc.vector.tensor_tensor(out=o, in0=a, in1=b, op=ALU.mult)
        return o

    def add(self, a, b):
        o = self.t()
        self.nc.vector.tensor_tensor(out=o, in0=a, in1=b, op=ALU.add)
        return o

    def sub(self, a, b):
        o = self.t()
        self.nc.vector.tensor_tensor(out=o, in0=a, in1=b, op=ALU.subtract)
        return o

    def mac(self, a, s, acc):
        """(a * s) + acc, s is a [B,1] AP scalar."""
        o = self.t()
        self.nc.vector.scalar_tensor_tensor(
            out=o, in0=a, scalar=s, in1=acc, op0=ALU.mult, op1=ALU.add
        )
        return o

    def dot3(self, ax, ay, az, bx, by, bz):
        o = self.mul(ax, bx)
        o = self.mac(ay, by, o)
        o = self.mac(az, bz, o)
        return o

    def cross3(self, ax, ay, az, bx, by, bz):
        """a x b -> 3 [B,1] tiles."""
        cx = self.sub(self.mul(ay, bz), self.mul(az, by))
        cy = self.sub(self.mul(az, bx), self.mul(ax, bz))
        cz = self.sub(self.mul(ax, by), self.mul(ay, bx))
        return cx, cy, cz


def axis_angle_R(nc, g, aa3, pfx, halfpi):
    R_ = g.rows
    """aa3: [R,3] axis-angle tile -> R [R,9] tile, R[l,i] at col l*3+i.

    R = c*I + s*K + (1-c) a a^T  (Rodrigues, matching reference)
    """
    pool = g.pool
    sq = pool.tile([R_, 3], F32, name=pfx + "aaR_sq")
    nc.vector.tensor_tensor(out=sq, in0=aa3, in1=aa3, op=ALU.mult)
    th2 = g.t()
    nc.vector.tensor_reduce(out=th2, in_=sq, axis=AX.X, op=ALU.add)
    theta = g.t()
    nc.scalar.activation(out=theta, in_=th2, func=ACTF.Sqrt)
    thm = g.t()
    nc.vector.tensor_scalar_max(out=thm, in0=theta, scalar1=1e-8)
    rth = g.t()
    nc.vector.reciprocal(out=rth, in_=thm)
    axis3 = pool.tile([R_, 3], F32, name=pfx + "aaR_axis")
    nc.vector.tensor_scalar_mul(out=axis3, in0=aa3, scalar1=rth)
    s = g.t()
    nc.scalar.activation(out=s, in_=theta, func=ACTF.Sin)
    c = g.t()
    nc.scalar.activation(out=c, in_=theta, func=ACTF.Sin, bias=halfpi)
    omc = g.t()
    nc.vector.tensor_scalar(
        out=omc, in0=c, scalar1=-1.0, scalar2=1.0, op0=ALU.mult, op1=ALU.add
    )
    ax, ay, az = axis3[:, 0:1], axis3[:, 1:2], axis3[:, 2:3]
    # diag: omc*a_i^2 + c
    asq = pool.tile([R_, 3], F32, name=pfx + "aaR_asq")
    nc.vector.tensor_tensor(out=asq, in0=axis3, in1=axis3, op=ALU.mult)
    R = pool.tile([R_, 9], F32, name=pfx + "aaR_R")
    dmul = pool.tile([R_, 3], F32, name=pfx + "aaR_dmul")
    nc.vector.tensor_scalar_mul(out=dmul, in0=asq, scalar1=omc)
    # s*a
    sa = pool.tile([R_, 3], F32, name=pfx + "aaR_sa")
    nc.vector.tensor_scalar_mul(out=sa, in0=axis3, scalar1=s)
    sax, say, saz = sa[:, 0:1], sa[:, 1:2], sa[:, 2:3]
    # off-diag products omc*ai*aj
    mxy = g.mul(g.mul(ax, ay), omc)
    mxz = g.mul(g.mul(ax, az), omc)
    myz = g.mul(g.mul(ay, az), omc)
    # assemble diag: R[l*4] = dmul_l + c
    for l in range(3):
        nc.vector.tensor_tensor(
            out=R[:, 4 * l:4 * l + 1], in0=dmul[:, l:l + 1], in1=c, op=ALU.add
        )
    nc.vector.tensor_tensor(out=R[:, 1:2], in0=mxy, in1=saz, op=ALU.subtract)  # R01
    nc.vector.tensor_tensor(out=R[:, 2:3], in0=mxz, in1=say, op=ALU.add)  # R02
    nc.vector.tensor_tensor(out=R[:, 3:4], in0=mxy, in1=saz, op=ALU.add)  # R10
    nc.vector.tensor_tensor(out=R[:, 5:6], in0=myz, in1=sax, op=ALU.subtract)  # R12
    nc.vector.tensor_tensor(out=R[:, 6:7], in0=mxz, in1=say, op=ALU.subtract)  # R20
    nc.vector.tensor_tensor(out=R[:, 7:8], in0=myz, in1=sax, op=ALU.add)  # R21
    return R


_ENG_ATTR = {
    "SP": "sync", "Pool": "gpsimd", "PE": "tensor",
    "DVE": "vector", "Activation": "scalar",
}


def _legalize_waits(nc):
    """This walrus accepts only one sync-wait slot per instruction; move extra
    waits onto same-engine NoOps inserted right before the instruction."""
    import concourse.mybir as _mybir

    def make_nop(engine):
        eng = getattr(nc, _ENG_ATTR[engine.name])
        bi = eng.nop(nofuse=True)
        mi = bi.ins
        for bb in nc.main_func.blocks:
            if bb.instructions and bb.instructions[-1].name == mi.name:
                bb.instructions.pop()
                break
        mi.engine = engine
        return mi

    for bb in nc.main_func.blocks:
        snapshot = list(bb.instructions)
        newlist = []
        changed = False
        for inst in snapshot:
            si = inst.sync_info
            waits = list(si.on_wait) if (si and si.on_wait) else []
            if (
                len(waits) > 1
                and not inst.name.startswith("barrier")
                and inst.engine is not None
                and getattr(inst.engine, "name", None) in _ENG_ATTR
            ):
                for w in waits[:-1]:
                    nop = make_nop(inst.engine)
                    nop.sync_info = _mybir.SyncInfo(on_wait=[w], on_update=[])
                    newlist.append(nop)
                inst.sync_info = _mybir.SyncInfo(
                    on_wait=[waits[-1]], on_update=list(si.on_update)
                )
                changed = True
            newlist.append(inst)
        if changed:
            bb.instructions[:] = newlist
    return nc


def build_graph():
    nc = bass.Bass(target_bir_lowering=False)

    x_p = nc.declare_dram_parameter("x_sh", [NCHUNK, 128, TPC * B], F32, isOutput=False)
    w_p = nc.declare_dram_parameter("w_sh", [NCHUNK, 128, TPC * NLAT], F32, isOutput=False)
    b_p = nc.declare_dram_parameter("bias_row", [1, NLAT + 2 * B], F32, isOutput=False)
    bas_p = nc.declare_dram_parameter("basis", [4, 128, 3, NCOL], F32, isOutput=False)
    tpl_p = nc.declare_dram_parameter("tmpl", [1, 3, NCOL], F32, isOutput=False)
    cam_p = nc.declare_dram_parameter("cam", [B, 12], F32, isOutput=False)
    out_p = nc.declare_dram_parameter("out", [B, 3, OUTW], F32, isOutput=True)

    ar_in = nc.dram_tensor("ar_in", [B, NLAT], F32)
    ar_out = nc.dram_tensor("ar_out", [B, NLAT], F32, addr_space="Shared")

    with tile.TileContext(nc) as tc:
        with (
            tc.tile_pool(name="consts", bufs=1) as consts,
            tc.tile_pool(name="latents", bufs=1) as latp,
            tc.tile_pool(name="geo", bufs=1) as geop,
            tc.tile_pool(name="planes", bufs=1) as planep,
            tc.tile_pool(name="bas", bufs=1) as basp,
            tc.tile_pool(name="dum", bufs=1, space="PSUM") as dum,
        ):
            b_sb = consts.tile([1, NLAT + 2 * B], F32)
            nc.sync.dma_start(out=b_sb, in_=b_p[:, :])
            ones8 = b_sb[:, NLAT:NLAT + B]          # value 1/NCORES, packed by host
            ones1 = b_sb[:, NLAT + B:NLAT + 2 * B]  # value 1.0, packed by host
            halfpi = consts.tile([128, 1], F32)
            nc.vector.memset(halfpi, HALF_PI)
            tmpl_sb = consts.tile([1, 3, NCOL], F32)
            nc.gpsimd.dma_start(out=tmpl_sb, in_=tpl_p[0])
            cam = consts.tile([B, 12], F32)
            nc.gpsimd.dma_start(out=cam, in_=cam_p[:, :])
            # basis prefetch: DMA engines stay busy through the AllReduce bubble
            bts = []
            for ki, (k0, kw) in enumerate(KSPL):
                bt = basp.tile([128, 3, NCOL], F32, name=f"bt{ki}")
                nc.gpsimd.dma_start(out=bt[:kw, :, :], in_=bas_p[ki, :kw, :, :])
                bts.append(bt)
            # PE matmuls carry a single sync-wait slot on this walrus; dummy
            # 1-wait matmuls make PE observe one dep before the real matmul.
            d1 = dum.tile([1, 1], F32)

            # ---------------- Phase 1: encoder GEMM (bf16) ----------------
            with (
                tc.tile_pool(name="xin", bufs=3) as xin,
                tc.tile_pool(name="wts", bufs=4) as wts,
                tc.tile_pool(name="encp", bufs=1, space="PSUM") as encp,
            ):
                pe = encp.tile([B, NLAT], F32, name="pe_enc", tag="pe_enc")
                for ci in range(NCHUNK):
                    x_c = xin.tile([128, TPC * B], F32)
                    nc.gpsimd.dma_start(out=x_c, in_=x_p[ci])
                    nc.tensor.matmul(
                        d1, lhsT=x_c[:, 0:1], rhs=x_c[:, 0:1],
                        start=True, stop=True, skip_group_check=True,
                    )
                    w_c = wts.tile([128, TPC * NLAT], F32)
                    weng = nc.sync if ci % 2 == 0 else nc.scalar
                    weng.dma_start(out=w_c, in_=w_p[ci])
                    for t in range(TPC):
                        nc.tensor.matmul(
                            pe,
                            lhsT=x_c[:, t * B:(t + 1) * B],
                            rhs=w_c[:, t * NLAT:(t + 1) * NLAT],
                            start=(ci == 0 and t == 0),
                            stop=False,
                        )
                nc.tensor.matmul(
                    pe, lhsT=ones8, rhs=b_sb[:, 0:NLAT], start=False, stop=True
                )
                lat1 = latp.tile([B, NLAT], F32)
                nc.vector.tensor_copy(out=lat1, in_=pe)
                nc.sync.dma_start(out=ar_in[:, :], in_=lat1)

            nc.gpsimd.collective_compute(
                "AllReduce",
                ALU.add,
                replica_groups=[list(range(NCORES))],
                ins=[ar_in.ap().opt()],
                outs=[ar_out.ap().opt()],
            )
            # lat: [64, 416]; cols 411:416 zeroed (transpose blocks read past 411)
            lat = latp.tile([B, 416], F32)
            nc.vector.memset(lat, 0.0)
            nc.sync.dma_start(out=lat[:, 0:NLAT], in_=ar_out[:, :])

            # ---------------- Phase 1.5: transpose shape params ----------------
            # DVE 32x32 block transposes: spT[ki][r, b] = lat[b, c0+r]; bf16 copies.
            spT = []
            for ki, (c0, kw) in enumerate(KSPL):
                st = latp.tile([kw, B], F32, name=f"spT{c0}", tag=f"spT{c0}")
                for pb in range(kw // 32):
                    for fb in range(B // 32):
                        nc.vector.transpose(
                            out=st[32 * pb:32 * pb + 32, 32 * fb:32 * fb + 32],
                            in_=lat[32 * fb:32 * fb + 32,
                                    c0 + 32 * pb:c0 + 32 * pb + 32],
                        )
                spT.append(st)

            # ---------------- Phase 2: blendshape GEMMs ----------------
            g = Geo(nc, geop)
            with tc.tile_pool(name="bpsum", bufs=1, space="PSUM") as bpsum:
                # geometry columns first: they feed the serial gaze chain
                pv_g = bpsum.tile([B, 3, NG], F32, name="pv_g", tag="pv_g")
                for p in range(3):
                    for ki, (k0, kw) in enumerate(KSPL):
                        if p == 0 and ki == 0:
                            # absorb the bt0-DMA wait so the real matmul only
                            # waits on the spT copy
                            nc.tensor.matmul(
                                d1, lhsT=bts[0][0:1, 0, 0:1], rhs=bts[0][0:1, 0, 0:1],
                                start=True, stop=True, skip_group_check=True,
                            )
                        nc.tensor.matmul(
                            pv_g[:, p, :],
                            lhsT=spT[ki][:kw, :],
                            rhs=bts[ki][:kw, p, 0:NG],
                            start=(ki == 0),
                            stop=False,
                        )
                    nc.tensor.matmul(
                        pv_g[:, p, :], lhsT=ones1, rhs=tmpl_sb[:, p, 0:NG],
                        start=False, stop=True,
                    )
                pv_s = []
                for p in range(3):
                    pv = bpsum.tile([B, VSH], F32, name=f"pv_s{p}", tag=f"pv_s{p}")
                    for ki, (k0, kw) in enumerate(KSPL):
                        nc.tensor.matmul(
                            pv,
                            lhsT=spT[ki][:kw, :],
                            rhs=bts[ki][:kw, p, NG:NCOL],
                            start=(ki == 0),
                            stop=False,
                        )
                    nc.tensor.matmul(
                        pv, lhsT=ones1, rhs=tmpl_sb[:, p, NG:NCOL],
                        start=False, stop=True,
                    )
                    pv_s.append(pv)

                # ---------- face transform scalars ----------
                aa_face = lat[:, 400:403]
                Rf = axis_angle_R(nc, g, aa_face, "f_", halfpi[:B, :])
                fs = g.t()  # face_scale = latent[551]+1
                nc.vector.tensor_scalar_add(out=fs, in0=lat[:, 406:407], scalar1=1.0)
                Rs = geop.tile([B, 9], F32)
                nc.vector.tensor_scalar_mul(out=Rs, in0=Rf, scalar1=fs)
                # offsets: off_i = face_t_i - sum_l vmean_l*Rs[l,i]
                off = geop.tile([B, 3], F32)
                for i in range(3):
                    t = g.mul(pv_g[:, 0, NG - 1:NG], Rs[:, i:i + 1])
                    t = g.mac(pv_g[:, 1, NG - 1:NG], Rs[:, 3 + i:4 + i], t)
                    t = g.mac(pv_g[:, 2, NG - 1:NG], Rs[:, 6 + i:7 + i], t)
                    nc.vector.tensor_tensor(
                        out=off[:, i:i + 1], in0=lat[:, 403 + i:404 + i], in1=t,
                        op=ALU.subtract,
                    )

                # ---------- fused rotation: rt = vs @ Rs + off ----------
                rt = planep.tile([B, 3, NCOL], F32)
                for i in range(3):  # geometry block (68 landmarks + lc/rc/fc)
                    nc.vector.tensor_scalar(
                        out=rt[:, i, 0:NG], in0=pv_g[:, 0, :],
                        scalar1=Rs[:, i:i + 1], scalar2=off[:, i:i + 1],
                        op0=ALU.mult, op1=ALU.add,
                    )
                    for l in (1, 2):
                        nc.vector.scalar_tensor_tensor(
                            out=rt[:, i, 0:NG], in0=pv_g[:, l, :],
                            scalar=Rs[:, 3 * l + i:3 * l + i + 1],
                            in1=rt[:, i, 0:NG],
                            op0=ALU.mult, op1=ALU.add,
                        )
                for i in range(3):  # vert shard block
                    nc.vector.tensor_scalar(
                        out=rt[:, i, NG:NCOL], in0=pv_s[0],
                        scalar1=Rs[:, i:i + 1], scalar2=off[:, i:i + 1],
                        op0=ALU.mult, op1=ALU.add,
                    )
                    for l in (1, 2):
                        nc.vector.scalar_tensor_tensor(
                            out=rt[:, i, NG:NCOL], in0=pv_s[l],
                            scalar=Rs[:, 3 * l + i:3 * l + i + 1],
                            in1=rt[:, i, NG:NCOL],
                            op0=ALU.mult, op1=ALU.add,
                        )

            # ---------- projection of the vert shard (GpSimd + 1 DVE recip) ----------
            img = planep.tile([B, 3, VSH], F32)
            for i in (2, 0, 1):  # z first (feeds the clamp chain)
                nc.vector.tensor_scalar(
                    out=img[:, i, :], in0=rt[:, 0, NG:NCOL],
                    scalar1=cam[:, 4 * i:4 * i + 1], scalar2=cam[:, 4 * i + 3:4 * i + 4],
                    op0=ALU.mult, op1=ALU.add,
                )
                for l in (1, 2):
                    nc.vector.scalar_tensor_tensor(
                        out=img[:, i, :], in0=rt[:, l, NG:NCOL],
                        scalar=cam[:, 4 * i + l:4 * i + l + 1], in1=img[:, i, :],
                        op0=ALU.mult, op1=ALU.add,
                    )
            az_ = planep.tile([B, VSH], F32)
            nc.scalar.activation(out=az_, in_=img[:, 2, :], func=ACTF.Abs)
            nc.gpsimd.tensor_scalar_max(out=az_, in0=az_, scalar1=1e-3)
            sg = planep.tile([B, VSH], F32)
            nc.gpsimd.tensor_scalar(
                out=sg, in0=img[:, 2, :], scalar1=0.0, scalar2=None, op0=ALU.is_ge
            )
            nc.gpsimd.tensor_scalar(
                out=sg, in0=sg, scalar1=2.0, scalar2=1.0,
                op0=ALU.mult, op1=ALU.subtract,
            )
            nc.gpsimd.tensor_tensor(out=sg, in0=sg, in1=az_, op=ALU.mult)
            nc.vector.reciprocal(out=az_, in_=sg)
            nc.gpsimd.tensor_tensor(
                out=img[:, 0, :], in0=img[:, 0, :], in1=az_, op=ALU.mult
            )
            nc.gpsimd.tensor_tensor(
                out=img[:, 1, :], in0=img[:, 1, :], in1=az_, op=ALU.mult
            )

            # ---------- eyeball rotations R2 (l/r stacked on partitions) ----------
            g2 = Geo(nc, geop, rows=128)
            aa2 = geop.tile([128, 3], F32)
            nc.vector.memset(aa2, 0.0)
            nc.vector.tensor_copy(out=aa2[0:B, 0:2], in_=lat[:, 407:409])
            nc.sync.dma_start(out=aa2[B:128, 0:2], in_=lat[:, 409:411])
            R2 = axis_angle_R(nc, g2, aa2, "e_", halfpi)
            # gaze = GAZE_DIR * R2[2,:]
            gz = geop.tile([128, 3], F32)
            nc.vector.tensor_scalar_mul(out=gz, in0=R2[:, 6:9], scalar1=GAZE_DIR)
            rg64 = geop.tile([B, 3], F32)
            nc.sync.dma_start(out=rg64, in_=gz[B:128, :])

            # transformed geometry verts: lc/rc/fc at geometry cols 68/69/70
            lc = [rt[:, i, 68:69] for i in range(3)]
            rc = [rt[:, i, 69:70] for i in range(3)]
            lg = [gz[0:B, i:i + 1] for i in range(3)]
            rg = [rg64[:, i:i + 1] for i in range(3)]

            # gaze intersection (Cramer)
            d = [g.sub(rc[i], lc[i]) for i in range(3)]
            c0 = lg
            c1 = []
            for i in range(3):
                o = g.t()
                nc.vector.tensor_scalar_mul(out=o, in0=rg[i], scalar1=-1.0)
                c1.append(o)
            c2 = list(g.cross3(*rg, *lg))
            # w = c1 x c2 ; det = c0.w ; num0 = d.w
            w = g.cross3(*c1, *c2)
            det = g.dot3(*c0, *w)
            num0 = g.dot3(*d, *w)
            # w2 = d x c2 ; num1 = c0.w2  (det with col1 replaced by d)
            w2 = g.cross3(*d, *c2)
            num1 = g.dot3(*c0, *w2)
            rdet = g.t()
            nc.vector.reciprocal(out=rdet, in_=det)
            sol0 = g.mul(num0, rdet)
            sol1 = g.mul(num1, rdet)
            # gp_l = l_c + sol0*lg ; gp_r = r_c + sol1*rg ; gp_mid
            gpl = geop.tile([B, 3], F32)
            gpr = geop.tile([B, 3], F32)
            gpm = geop.tile([B, 3], F32)
            for i in range(3):
                nc.vector.scalar_tensor_tensor(
                    out=gpl[:, i:i + 1], in0=lg[i], scalar=sol0,
                    in1=lc[i], op0=ALU.mult, op1=ALU.add,
                )
                nc.vector.scalar_tensor_tensor(
                    out=gpr[:, i:i + 1], in0=rg[i], scalar=sol1,
                    in1=rc[i], op0=ALU.mult, op1=ALU.add,
                )
            nc.vector.tensor_tensor(out=gpm, in0=gpl, in1=gpr, op=ALU.add)
            nc.vector.tensor_scalar_mul(out=gpm, in0=gpm, scalar1=0.5)
            dff = geop.tile([B, 3], F32)
            nc.vector.tensor_tensor(out=dff, in0=gpl, in1=gpr, op=ALU.subtract)
            nc.vector.tensor_tensor(out=dff, in0=dff, in1=dff, op=ALU.mult)
            d2 = g.t()
            nc.vector.tensor_reduce(out=d2, in_=dff, axis=AX.X, op=ALU.add)
            dist = g.t()
            nc.scalar.activation(out=dist, in_=d2, func=ACTF.Sqrt)
            # far points l_c + 1000*lg
            farl = geop.tile([B, 3], F32)
            farr = geop.tile([B, 3], F32)
            for i in range(3):
                nc.vector.scalar_tensor_tensor(
                    out=farl[:, i:i + 1], in0=lg[i], scalar=1000.0,
                    in1=lc[i], op0=ALU.mult, op1=ALU.add,
                )
                nc.vector.scalar_tensor_tensor(
                    out=farr[:, i:i + 1], in0=rg[i], scalar=1000.0,
                    in1=rc[i], op0=ALU.mult, op1=ALU.add,
                )

            # ---------- tail assembly [B, 3, 11] ----------
            tail = geop.tile([B, 3, 11], F32)
            def _cp(k, out, in_):
                e = k % 3
                if e == 0:
                    nc.vector.tensor_copy(out=out, in_=in_)
                elif e == 1:
                    nc.scalar.copy(out=out, in_=in_)
                else:
                    nc.gpsimd.tensor_copy(out=out, in_=in_)

            _cp(0, tail[:, :, 0:1], rt[:, :, 68:69])   # l_c
            _cp(1, tail[:, :, 1:2], rt[:, :, 69:70])   # r_c
            _cp(2, tail[:, :, 2:3], rt[:, :, 70:71])   # face_centre
            _cp(0, tail[:, :, 3:4], gpl)
            _cp(1, tail[:, :, 4:5], gpr)
            _cp(2, tail[:, :, 5:6], gpm)
            _cp(0, tail[:, :, 6:7], farl)
            _cp(1, tail[:, :, 7:8], farr)
            _cp(2, tail[:, :, 8:9], gz[0:B, :])
            _cp(0, tail[:, :, 9:10], rg64)
            for i in range(3):
                _cp(i + 1, tail[:, i, 10:11], dist)

            # ---------- output DMAs ----------
            nc.sync.dma_start(out=out_p[:, :, 0:VSH], in_=rt[:, :, NG:NCOL])
            nc.scalar.dma_start(out=out_p[:, :, VSH:2 * VSH], in_=img)
            nc.gpsimd.dma_start(out=out_p[:, :, 2 * VSH:2 * VSH + 68], in_=rt[:, :, 0:68])
            nc.sync.dma_start(out=out_p[:, :, 2 * VSH + 68:OUTW], in_=tail)
    _legalize_waits(nc)
    return nc


def _prep(inputs):
    x = np.ascontiguousarray(inputs["x"].reshape(B, DIN), dtype=np.float32)
    enc_W = np.asarray(inputs["enc_W"], dtype=np.float32)
    basis = np.asarray(inputs["shape_basis"], dtype=np.float32)  # [400, V, 3]
    tmpl = np.asarray(inputs["v_template"], dtype=np.float32)    # [V, 3]
    cam = np.ascontiguousarray(
        np.asarray(inputs["camera_parameters"], dtype=np.float32).reshape(B, 12)
    )
    lm = np.asarray(inputs["landmarks"])
    mlm = np.asarray(inputs["masked_landmarks"])
    fmask = np.asarray(inputs["face_mask"])
    lmask = np.asarray(inputs["left_eyeball_mask"])
    rmask = np.asarray(inputs["right_eyeball_mask"])

    bias_row = np.concatenate([
        np.asarray(inputs["enc_b"], dtype=np.float32).reshape(1, LAT)[:, :400],
        np.asarray(inputs["enc_b"], dtype=np.float32).reshape(1, LAT)[:, 545:556],
        np.full((1, B), 1.0 / NCORES, np.float32),
        np.ones((1, B), np.float32),
    ], axis=1)

    # geometry columns [400, 72, 3] and [72, 3]
    fl_v = fmask[mlm]  # vert ids of the masked landmarks (in 0:3500)
    idx4 = lm[[19, 22, 25, 28]]
    idx2 = lm[[14, 18]]
    bG = np.empty((400, NG, 3), np.float32)
    bG[:, 0:68] = basis[:, fl_v, :]
    bG[:, 68] = basis[:, lmask, :].mean(axis=1)
    bG[:, 69] = basis[:, rmask, :].mean(axis=1)
    bG[:, 70] = basis[:, idx4, :].sum(axis=1) / 8.0 + basis[:, idx2, :].sum(axis=1) / 4.0
    bG[:, 71] = basis.mean(axis=1)
    tG = np.empty((NG, 3), np.float32)
    tG[0:68] = tmpl[fl_v]
    tG[68] = tmpl[lmask].mean(axis=0)
    tG[69] = tmpl[rmask].mean(axis=0)
    tG[70] = tmpl[idx4].sum(axis=0) / 8.0 + tmpl[idx2].sum(axis=0) / 4.0
    tG[71] = tmpl.mean(axis=0)

    # used encoder columns
    wcols = np.concatenate([enc_W[:, 0:400], enc_W[:, 545:556]], axis=1)  # [DIN, 411]

    in_maps = []
    for c in range(NCORES):
        k0 = c * KSH
        # x shard: [KSH, B] -> [21, 128, 7*64] (k = ci*896 + t*128 + p)
        xs = x[:, k0:k0 + KSH].T.reshape(NCHUNK, TPC, 128, B)
        xs = np.ascontiguousarray(xs.transpose(0, 2, 1, 3).reshape(NCHUNK, 128, TPC * B))
        ws = wcols[k0:k0 + KSH].reshape(NCHUNK, TPC, 128, NLAT)
        ws = np.ascontiguousarray(ws.transpose(0, 2, 1, 3).reshape(NCHUNK, 128, TPC * NLAT))
        # vert shard columns (pad past 3500 with zeros)
        v0 = c * VSH
        nv = max(0, min(VSH, VM - v0))
        bS = np.zeros((400, VSH, 3), np.float32)
        bS[:, :nv] = basis[:, v0:v0 + nv, :]
        tS = np.zeros((VSH, 3), np.float32)
        tS[:nv] = tmpl[v0:v0 + nv]
        bfull = np.concatenate([bG, bS], axis=1).transpose(0, 2, 1)  # [400, 3, 520]
        bpad = np.zeros((4 * 128, 3, NCOL), np.float32)
        bpad[0:400] = bfull
        bpad = bpad.reshape(4, 128, 3, NCOL)
        tfull = np.concatenate([tG, tS], axis=0).T.reshape(1, 3, NCOL)  # [1, 3, 520]
        in_maps.append({
            "x_sh": xs,
            "w_sh": ws,
            "bias_row": bias_row,
            "basis": bpad,
            "tmpl": np.ascontiguousarray(tfull),
            "cam": cam,
        })
    return in_maps


def _assemble(results):
    out = np.empty((B, NOUT, 3), np.float32)
    for c in range(NCORES):
        o = results[c]["out"]  # [B, 3, OUTW]
        v0 = c * VSH
        nv = max(0, min(VSH, VM - v0))
        if nv > 0:
            out[:, v0:v0 + nv, :] = o[:, :, 0:nv].transpose(0, 2, 1)
            out[:, VM + v0:VM + v0 + nv, :] = o[:, :, VSH:VSH + nv].transpose(0, 2, 1)
    o0 = results[0]["out"]
    out[:, 2 * VM:2 * VM + 68, :] = o0[:, :, 2 * VSH:2 * VSH + 68].transpose(0, 2, 1)
    out[:, 2 * VM + 68:NOUT, :] = o0[:, :, 2 * VSH + 68:OUTW].transpose(0, 2, 1)
    return out


def _run(inputs, trace=False):
    in_maps = _prep(inputs)
    nc = build_graph()
    res = run_bass_kernel_spmd(
        nc, in_maps, core_ids=list(range(NCORES)), trace=trace
    )
    out = _assemble(res.results)
    return np.ascontiguousarray(out), res


def kernel(**inputs):
    out, _ = _run(inputs, trace=False)
    return out


# revision 9
# speedup vs baseline: 1.9233x; 1.2473x over previous
"""Trainium2 Bass kernel for nn_Autoencoder_65223373357102 (FLAME-style autoencoder).

Strategy (v2):
  Phase 1 (8-way tensor parallel): encoder GEMM [64,150528]@[150528,411] in fp32,
  sharded along the input-feature axis. Only the 411 *used* latent columns are
  computed (shape_p 0:400 + geometry latents 545:556; cols 400:545 are never read
  by the reference). Each core multiplies its x/W shard, adds bias via a K=1
  matmul, and AllReduces the [64,411] fp32 latent (105 KB).
  Phase 2 (vertex-sharded): the rotated eye vertices are dead code (face_mask
  0:3500 is disjoint from the eyeball masks), and eye centres / face centre /
  landmarks are affine images of precomputed basis mean-columns. So each core
  runs an fp32 blendshape GEMM over [72 geometry columns | its 448-column shard of
  the 3500 face verts], applies the fused face rotation, projects its shard, and
  runs the tiny per-batch gaze/Cramer chain. The host concatenates the 8 shard
  outputs; geometry/landmark sections are replicated and taken from core 0.
"""
import sys
import types

sys.path.insert(0, "/opt/trn_rl_repo")

import numpy as np


def _ensure_ntff_hook():
    """Provide antenv.axon_hooks + install the ctypes NTFF profile hook so
    run_bass_kernel_spmd(trace=True) can pull a neuron-profile under axon."""
    name = "antenv.axon_hooks"
    if name not in sys.modules:
        mod = types.ModuleType(name)
        mod._HOOK = None

        def set_axon_ntff_profile_hook(hook):
            mod._HOOK = hook

        def get_axon_ntff_profile_hook():
            return mod._HOOK

        mod.set_axon_ntff_profile_hook = set_axon_ntff_profile_hook
        mod.get_axon_ntff_profile_hook = get_axon_ntff_profile_hook
        sys.modules[name] = mod
        try:
            import antenv

            antenv.axon_hooks = mod
        except ImportError:
            pass
    mod = sys.modules[name]
    if mod.get_axon_ntff_profile_hook() is None:
        try:
            from trn_agent_boot.trn_boot import _ntff_profile_via_ctypes

            hook = _ntff_profile_via_ctypes("/opt/axon/libaxon_pjrt.so")
            if hook is not None:
                mod.set_axon_ntff_profile_hook(hook)
        except Exception:
            pass


_ensure_ntff_hook()

from concourse import bass, mybir, tile
from concourse.bass_utils import run_bass_kernel_spmd

F32 = mybir.dt.float32
ALU = mybir.AluOpType
ACTF = mybir.ActivationFunctionType
AX = mybir.AxisListType

B = 64
V = 5023
VM = 3500
LAT = 556
NLAT = 411          # used latent cols: 0:400 (shape) + 545:556 (geometry)
DIN = 3 * 224 * 224  # 150528
NCORES = 8
KSH = DIN // NCORES  # 18816
TPC = 7              # k-tiles per chunk
NCHUNK = KSH // (128 * TPC)  # 21
NOUT = 2 * VM + 68 + 11  # 7079
GAZE_DIR = -1.0
HALF_PI = 1.5707963267948966

NG = 72              # geometry columns: 68 landmarks | l_mean | r_mean | fc | vmean
VSH = 448            # face-vert shard columns per core (8*448 = 3584 >= 3500)
NCOL = NG + VSH      # 520
KSPL = [(0, 128), (128, 128), (256, 128), (384, 32)]  # shape-K tiles (400 pad 416)
OUTW = VSH + VSH + 68 + 11  # 975 per-core output cols


class Geo:
    """Helper for tiny per-batch scalar ops on [rows,1] tiles."""

    _uid = [0]

    def __init__(self, nc, pool, rows=B):
        self.nc = nc
        self.pool = pool
        self.rows = rows

    def t(self, cols=1):
        Geo._uid[0] += 1
        return self.pool.tile([self.rows, cols], F32, name=f"g{Geo._uid[0]}_{cols}")

    def mul(self, a, b):
        o = self.t()
        self.nc.vector.tensor_tensor(out=o, in0=a, in1=b, op=ALU.mult)
        return o

    def add(self, a, b):
        o = self.t()
        self.nc.vector.tensor_tensor(out=o, in0=a, in1=b, op=ALU.add)
        return o

    def sub(self, a, b):
        o = self.t()
        self.nc.vector.tensor_tensor(out=o, in0=a, in1=b, op=ALU.subtract)
        return o

    def mac(self, a, s, acc):
        """(a * s) + acc, s is a [B,1] AP scalar."""
        o = self.t()
        self.nc.vector.scalar_tensor_tensor(
            out=o, in0=a, scalar=s, in1=acc, op0=ALU.mult, op1=ALU.add
        )
        return o

    def dot3(self, ax, ay, az, bx, by, bz):
        o = self.mul(ax, bx)
        o = self.mac(ay, by, o)
        o = self.mac(az, bz, o)
        return o

    def cross3(self, ax, ay, az, bx, by, bz):
        """a x b -> 3 [B,1] tiles."""
        cx = self.sub(self.mul(ay, bz), self.mul(az, by))
        cy = self.sub(self.mul(az, bx), self.mul(ax, bz))
        cz = self.sub(self.mul(ax, by), self.mul(ay, bx))
        return cx, cy, cz


def axis_angle_R(nc, g, aa3, pfx, halfpi):
    R_ = g.rows
    """aa3: [R,3] axis-angle tile -> R [R,9] tile, R[l,i] at col l*3+i.

    R = c*I + s*K + (1-c) a a^T  (Rodrigues, matching reference)
    """
    pool = g.pool
    sq = pool.tile([R_, 3], F32, name=pfx + "aaR_sq")
    nc.vector.tensor_tensor(out=sq, in0=aa3, in1=aa3, op=ALU.mult)
    th2 = g.t()
    nc.vector.tensor_reduce(out=th2, in_=sq, axis=AX.X, op=ALU.add)
    theta = g.t()
    nc.scalar.activation(out=theta, in_=th2, func=ACTF.Sqrt)
    thm = g.t()
    nc.vector.tensor_scalar_max(out=thm, in0=theta, scalar1=1e-8)
    rth = g.t()
    nc.vector.reciprocal(out=rth, in_=thm)
    axis3 = pool.tile([R_, 3], F32, name=pfx + "aaR_axis")
    nc.vector.tensor_scalar_mul(out=axis3, in0=aa3, scalar1=rth)
    s = g.t()
    nc.scalar.activation(out=s, in_=theta, func=ACTF.Sin)
    c = g.t()
    nc.scalar.activation(out=c, in_=theta, func=ACTF.Sin, bias=halfpi)
    omc = g.t()
    nc.vector.tensor_scalar(
        out=omc, in0=c, scalar1=-1.0, scalar2=1.0, op0=ALU.mult, op1=ALU.add
    )
    ax, ay, az = axis3[:, 0:1], axis3[:, 1:2], axis3[:, 2:3]
    # diag: omc*a_i^2 + c
    asq = pool.tile([R_, 3], F32, name=pfx + "aaR_asq")
    nc.vector.tensor_tensor(out=asq, in0=axis3, in1=axis3, op=ALU.mult)
    R = pool.tile([R_, 9], F32, name=pfx + "aaR_R")
    dmul = pool.tile([R_, 3], F32, name=pfx + "aaR_dmul")
    nc.vector.tensor_scalar_mul(out=dmul, in0=asq, scalar1=omc)
    # s*a
    sa = pool.tile([R_, 3], F32, name=pfx + "aaR_sa")
    nc.vector.tensor_scalar_mul(out=sa, in0=axis3, scalar1=s)
    sax, say, saz = sa[:, 0:1], sa[:, 1:2], sa[:, 2:3]
    # off-diag products omc*ai*aj
    mxy = g.mul(g.mul(ax, ay), omc)
    mxz = g.mul(g.mul(ax, az), omc)
    myz = g.mul(g.mul(ay, az), omc)
    # assemble diag: R[l*4] = dmul_l + c
    for l in range(3):
        nc.vector.tensor_tensor(
            out=R[:, 4 * l:4 * l + 1], in0=dmul[:, l:l + 1], in1=c, op=ALU.add
        )
    nc.vector.tensor_tensor(out=R[:, 1:2], in0=mxy, in1=saz, op=ALU.subtract)  # R01
    nc.vector.tensor_tensor(out=R[:, 2:3], in0=mxz, in1=say, op=ALU.add)  # R02
    nc.vector.tensor_tensor(out=R[:, 3:4], in0=mxy, in1=saz, op=ALU.add)  # R10
    nc.vector.tensor_tensor(out=R[:, 5:6], in0=myz, in1=sax, op=ALU.subtract)  # R12
    nc.vector.tensor_tensor(out=R[:, 6:7], in0=mxz, in1=say, op=ALU.subtract)  # R20
    nc.vector.tensor_tensor(out=R[:, 7:8], in0=myz, in1=sax, op=ALU.add)  # R21
    return R


_ENG_ATTR = {
    "SP": "sync", "Pool": "gpsimd", "PE": "tensor",
    "DVE": "vector", "Activation": "scalar",
}


def _legalize_waits(nc):
    """This walrus accepts only one sync-wait slot per instruction; move extra
    waits onto same-engine NoOps inserted right before the instruction."""
    import concourse.mybir as _mybir

    def make_nop(engine):
        eng = getattr(nc, _ENG_ATTR[engine.name])
        bi = eng.nop(nofuse=True)
        mi = bi.ins
        for bb in nc.main_func.blocks:
            if bb.instructions and bb.instructions[-1].name == mi.name:
                bb.instructions.pop()
                break
        mi.engine = engine
        return mi

    for bb in nc.main_func.blocks:
        snapshot = list(bb.instructions)
        newlist = []
        changed = False
        for inst in snapshot:
            si = inst.sync_info
            waits = list(si.on_wait) if (si and si.on_wait) else []
            if (
                len(waits) > 1
                and not inst.name.startswith("barrier")
                and inst.engine is not None
                and getattr(inst.engine, "name", None) in _ENG_ATTR
            ):
                for w in waits[:-1]:
                    nop = make_nop(inst.engine)
                    nop.sync_info = _mybir.SyncInfo(on_wait=[w], on_update=[])
                    newlist.append(nop)
                inst.sync_info = _mybir.SyncInfo(
                    on_wait=[waits[-1]], on_update=list(si.on_update)
                )
                changed = True
            newlist.append(inst)
        if changed:
            bb.instructions[:] = newlist
    return nc


def build_graph():
    nc = bass.Bass(target_bir_lowering=False)

    x_p = nc.declare_dram_parameter("x_sh", [NCHUNK, 128, TPC * B], F32, isOutput=False)
    w_p = nc.declare_dram_parameter("w_sh", [NCHUNK, 128, TPC * NLAT], F32, isOutput=False)
    b_p = nc.declare_dram_parameter("bias_row", [1, NLAT + 2 * B], F32, isOutput=False)
    bas_p = nc.declare_dram_parameter("basis", [4, 128, 3, NCOL], F32, isOutput=False)
    tpl_p = nc.declare_dram_parameter("tmpl", [1, 3, NCOL], F32, isOutput=False)
    cam_p = nc.declare_dram_parameter("cam", [B, 12], F32, isOutput=False)
    out_p = nc.declare_dram_parameter("out", [B, 3, OUTW], F32, isOutput=True)

    ar_in = nc.dram_tensor("ar_in", [B, NLAT], F32)
    ar_out = nc.dram_tensor("ar_out", [B, NLAT], F32, addr_space="Shared")

    with tile.TileContext(nc) as tc:
        with (
            tc.tile_pool(name="consts", bufs=1) as consts,
            tc.tile_pool(name="latents", bufs=1) as latp,
            tc.tile_pool(name="geo", bufs=1) as geop,
            tc.tile_pool(name="planes", bufs=1) as planep,
            tc.tile_pool(name="bas", bufs=1) as basp,
            tc.tile_pool(name="dum", bufs=1, space="PSUM") as dum,
        ):
            b_sb = consts.tile([1, NLAT + 2 * B], F32)
            nc.sync.dma_start(out=b_sb, in_=b_p[:, :])
            ones8 = b_sb[:, NLAT:NLAT + B]          # value 1/NCORES, packed by host
            ones1 = b_sb[:, NLAT + B:NLAT + 2 * B]  # value 1.0, packed by host
            halfpi = consts.tile([128, 1], F32)
            nc.vector.memset(halfpi, HALF_PI)
            # PE matmuls carry a single sync-wait slot on this walrus; dummy
            # 1-wait matmuls make PE observe one dep before the real matmul.
            d1 = dum.tile([1, 1], F32)

            # ---------------- Phase 1: encoder GEMM (bf16) ----------------
            with (
                tc.tile_pool(name="xin", bufs=3) as xin,
                tc.tile_pool(name="wts", bufs=4) as wts,
                tc.tile_pool(name="encp", bufs=1, space="PSUM") as encp,
            ):
                pe = encp.tile([B, NLAT], F32, name="pe_enc", tag="pe_enc")
                for ci in range(NCHUNK):
                    x_c = xin.tile([128, TPC * B], F32)
                    nc.gpsimd.dma_start(out=x_c, in_=x_p[ci])
                    w_c = wts.tile([128, TPC * NLAT], F32)
                    weng = nc.sync if ci % 2 == 0 else nc.scalar
                    weng.dma_start(out=w_c, in_=w_p[ci])
                    for t in range(TPC):
                        nc.tensor.matmul(
                            pe,
                            lhsT=x_c[:, t * B:(t + 1) * B],
                            rhs=w_c[:, t * NLAT:(t + 1) * NLAT],
                            start=(ci == 0 and t == 0),
                            stop=False,
                        )
                nc.tensor.matmul(
                    pe, lhsT=ones8, rhs=b_sb[:, 0:NLAT], start=False, stop=True
                )
                lat1 = latp.tile([B, NLAT], F32)
                nc.vector.tensor_copy(out=lat1, in_=pe)
                nc.sync.dma_start(out=ar_in[:, :], in_=lat1)

            # prefetch phase-2 constants behind the x chunks; they only need to
            # land before the AllReduce completes
            tmpl_sb = consts.tile([1, 3, NCOL], F32)
            nc.gpsimd.dma_start(out=tmpl_sb, in_=tpl_p[0])
            cam = consts.tile([B, 12], F32)
            nc.gpsimd.dma_start(out=cam, in_=cam_p[:, :])
            bts = []
            for ki, (k0, kw) in enumerate(KSPL):
                bt = basp.tile([128, 3, NCOL], F32, name=f"bt{ki}")
                nc.gpsimd.dma_start(out=bt[:kw, :, :], in_=bas_p[ki, :kw, :, :])
                bts.append(bt)

            nc.gpsimd.collective_compute(
                "AllReduce",
                ALU.add,
                replica_groups=[list(range(NCORES))],
                ins=[ar_in.ap().opt()],
                outs=[ar_out.ap().opt()],
            )
            # lat: [64, 416]; cols 411:416 zeroed (transpose blocks read past 411)
            lat = latp.tile([B, 416], F32)
            nc.vector.memset(lat, 0.0)
            nc.sync.dma_start(out=lat[:, 0:NLAT], in_=ar_out[:, :])

            # ---------------- Phase 1.5: transpose shape params ----------------
            # DVE 32x32 block transposes: spT[ki][r, b] = lat[b, c0+r]; bf16 copies.
            spT = []
            for ki, (c0, kw) in enumerate(KSPL):
                st = latp.tile([kw, B], F32, name=f"spT{c0}", tag=f"spT{c0}")
                for pb in range(kw // 32):
                    for fb in range(B // 32):
                        nc.vector.transpose(
                            out=st[32 * pb:32 * pb + 32, 32 * fb:32 * fb + 32],
                            in_=lat[32 * fb:32 * fb + 32,
                                    c0 + 32 * pb:c0 + 32 * pb + 32],
                        )
                spT.append(st)

            # ---------------- Phase 2: blendshape GEMMs ----------------
            g = Geo(nc, geop)
            # face rotation scalars + eyeball rotations first: they only need
            # lat and feed the longest serial chains
            aa_face = lat[:, 400:403]
            Rf = axis_angle_R(nc, g, aa_face, "f_", halfpi[:B, :])
            fs = g.t()  # face_scale = latent[551]+1
            nc.vector.tensor_scalar_add(out=fs, in0=lat[:, 406:407], scalar1=1.0)
            Rs = geop.tile([B, 9], F32)
            nc.vector.tensor_scalar_mul(out=Rs, in0=Rf, scalar1=fs)
            g2 = Geo(nc, geop, rows=128)
            aa2 = geop.tile([128, 3], F32)
            nc.vector.memset(aa2, 0.0)
            nc.vector.tensor_copy(out=aa2[0:B, 0:2], in_=lat[:, 407:409])
            nc.sync.dma_start(out=aa2[B:128, 0:2], in_=lat[:, 409:411])
            R2 = axis_angle_R(nc, g2, aa2, "e_", halfpi)
            # gaze = GAZE_DIR * R2[2,:]
            gz = geop.tile([128, 3], F32)
            nc.vector.tensor_scalar_mul(out=gz, in0=R2[:, 6:9], scalar1=GAZE_DIR)
            rg64 = geop.tile([B, 3], F32)
            nc.sync.dma_start(out=rg64, in_=gz[B:128, :])
            with tc.tile_pool(name="bpsum", bufs=1, space="PSUM") as bpsum:
                # geometry columns first: they feed the serial gaze chain
                pv_g = bpsum.tile([B, 3, NG], F32, name="pv_g", tag="pv_g")
                for p in range(3):
                    for ki, (k0, kw) in enumerate(KSPL):
                        if p == 0 and ki == 0:
                            # absorb the bt0-DMA wait so the real matmul only
                            # waits on the spT copy
                            nc.tensor.matmul(
                                d1, lhsT=bts[0][0:1, 0, 0:1], rhs=bts[0][0:1, 0, 0:1],
                                start=True, stop=True, skip_group_check=True,
                            )
                        nc.tensor.matmul(
                            pv_g[:, p, :],
                            lhsT=spT[ki][:kw, :],
                            rhs=bts[ki][:kw, p, 0:NG],
                            start=(ki == 0),
                            stop=False,
                        )
                    nc.tensor.matmul(
                        pv_g[:, p, :], lhsT=ones1, rhs=tmpl_sb[:, p, 0:NG],
                        start=False, stop=True,
                    )
                pv_s = []
                for p in range(3):
                    pv = bpsum.tile([B, VSH], F32, name=f"pv_s{p}", tag=f"pv_s{p}")
                    for ki, (k0, kw) in enumerate(KSPL):
                        nc.tensor.matmul(
                            pv,
                            lhsT=spT[ki][:kw, :],
                            rhs=bts[ki][:kw, p, NG:NCOL],
                            start=(ki == 0),
                            stop=False,
                        )
                    nc.tensor.matmul(
                        pv, lhsT=ones1, rhs=tmpl_sb[:, p, NG:NCOL],
                        start=False, stop=True,
                    )
                    pv_s.append(pv)

                # offsets: off_i = face_t_i - sum_l vmean_l*Rs[l,i]
                off = geop.tile([B, 3], F32)
                for i in range(3):
                    t = g.mul(pv_g[:, 0, NG - 1:NG], Rs[:, i:i + 1])
                    t = g.mac(pv_g[:, 1, NG - 1:NG], Rs[:, 3 + i:4 + i], t)
                    t = g.mac(pv_g[:, 2, NG - 1:NG], Rs[:, 6 + i:7 + i], t)
                    nc.vector.tensor_tensor(
                        out=off[:, i:i + 1], in0=lat[:, 403 + i:404 + i], in1=t,
                        op=ALU.subtract,
                    )

                # ---------- fused rotation: rt = vs @ Rs + off ----------
                rt = planep.tile([B, 3, NCOL], F32)
                for i in range(3):  # geometry block (68 landmarks + lc/rc/fc)
                    nc.vector.tensor_scalar(
                        out=rt[:, i, 0:NG], in0=pv_g[:, 0, :],
                        scalar1=Rs[:, i:i + 1], scalar2=off[:, i:i + 1],
                        op0=ALU.mult, op1=ALU.add,
                    )
                    for l in (1, 2):
                        nc.vector.scalar_tensor_tensor(
                            out=rt[:, i, 0:NG], in0=pv_g[:, l, :],
                            scalar=Rs[:, 3 * l + i:3 * l + i + 1],
                            in1=rt[:, i, 0:NG],
                            op0=ALU.mult, op1=ALU.add,
                        )
                for i in range(3):  # vert shard block
                    nc.vector.tensor_scalar(
                        out=rt[:, i, NG:NCOL], in0=pv_s[0],
                        scalar1=Rs[:, i:i + 1], scalar2=off[:, i:i + 1],
                        op0=ALU.mult, op1=ALU.add,
                    )
                    for l in (1, 2):
                        nc.vector.scalar_tensor_tensor(
                            out=rt[:, i, NG:NCOL], in0=pv_s[l],
                            scalar=Rs[:, 3 * l + i:3 * l + i + 1],
                            in1=rt[:, i, NG:NCOL],
                            op0=ALU.mult, op1=ALU.add,
                        )

            # ---------- projection of the vert shard (GpSimd + 1 DVE recip) ----------
            img = planep.tile([B, 3, VSH], F32)
            for i in (2, 0, 1):  # z first (feeds the clamp chain)
                nc.vector.tensor_scalar(
                    out=img[:, i, :], in0=rt[:, 0, NG:NCOL],
                    scalar1=cam[:, 4 * i:4 * i + 1], scalar2=cam[:, 4 * i + 3:4 * i + 4],
                    op0=ALU.mult, op1=ALU.add,
                )
                for l in (1, 2):
                    nc.vector.scalar_tensor_tensor(
                        out=img[:, i, :], in0=rt[:, l, NG:NCOL],
                        scalar=cam[:, 4 * i + l:4 * i + l + 1], in1=img[:, i, :],
                        op0=ALU.mult, op1=ALU.add,
                    )
            az_ = planep.tile([B, VSH], F32)
            nc.scalar.activation(out=az_, in_=img[:, 2, :], func=ACTF.Abs)
            nc.vector.tensor_scalar_max(out=az_, in0=az_, scalar1=1e-3)
            sg = planep.tile([B, VSH], F32)
            nc.vector.tensor_scalar(
                out=sg, in0=img[:, 2, :], scalar1=0.0, scalar2=None, op0=ALU.is_ge
            )
            nc.vector.tensor_scalar(
                out=sg, in0=sg, scalar1=2.0, scalar2=1.0,
                op0=ALU.mult, op1=ALU.subtract,
            )
            nc.vector.tensor_tensor(out=sg, in0=sg, in1=az_, op=ALU.mult)
            nc.vector.reciprocal(out=az_, in_=sg)
            nc.vector.tensor_tensor(
                out=img[:, 0, :], in0=img[:, 0, :], in1=az_, op=ALU.mult
            )
            nc.vector.tensor_tensor(
                out=img[:, 1, :], in0=img[:, 1, :], in1=az_, op=ALU.mult
            )

            # transformed geometry verts: lc/rc/fc at geometry cols 68/69/70
            lc = [rt[:, i, 68:69] for i in range(3)]
            rc = [rt[:, i, 69:70] for i in range(3)]
            lg = [gz[0:B, i:i + 1] for i in range(3)]
            rg = [rg64[:, i:i + 1] for i in range(3)]

            # gaze intersection (Cramer)
            d = [g.sub(rc[i], lc[i]) for i in range(3)]
            c0 = lg
            c1 = []
            for i in range(3):
                o = g.t()
                nc.vector.tensor_scalar_mul(out=o, in0=rg[i], scalar1=-1.0)
                c1.append(o)
            c2 = list(g.cross3(*rg, *lg))
            # w = c1 x c2 ; det = c0.w ; num0 = d.w
            w = g.cross3(*c1, *c2)
            det = g.dot3(*c0, *w)
            num0 = g.dot3(*d, *w)
            # w2 = d x c2 ; num1 = c0.w2  (det with col1 replaced by d)
            w2 = g.cross3(*d, *c2)
            num1 = g.dot3(*c0, *w2)
            rdet = g.t()
            nc.vector.reciprocal(out=rdet, in_=det)
            sol0 = g.mul(num0, rdet)
            sol1 = g.mul(num1, rdet)
            # gp_l = l_c + sol0*lg ; gp_r = r_c + sol1*rg ; gp_mid
            gpl = geop.tile([B, 3], F32)
            gpr = geop.tile([B, 3], F32)
            gpm = geop.tile([B, 3], F32)
            for i in range(3):
                nc.vector.scalar_tensor_tensor(
                    out=gpl[:, i:i + 1], in0=lg[i], scalar=sol0,
                    in1=lc[i], op0=ALU.mult, op1=ALU.add,
                )
                nc.vector.scalar_tensor_tensor(
                    out=gpr[:, i:i + 1], in0=rg[i], scalar=sol1,
                    in1=rc[i], op0=ALU.mult, op1=ALU.add,
                )
            nc.vector.tensor_tensor(out=gpm, in0=gpl, in1=gpr, op=ALU.add)
            nc.vector.tensor_scalar_mul(out=gpm, in0=gpm, scalar1=0.5)
            dff = geop.tile([B, 3], F32)
            nc.vector.tensor_tensor(out=dff, in0=gpl, in1=gpr, op=ALU.subtract)
            nc.vector.tensor_tensor(out=dff, in0=dff, in1=dff, op=ALU.mult)
            d2 = g.t()
            nc.vector.tensor_reduce(out=d2, in_=dff, axis=AX.X, op=ALU.add)
            dist = g.t()
            nc.scalar.activation(out=dist, in_=d2, func=ACTF.Sqrt)
            # far points l_c + 1000*lg
            farl = geop.tile([B, 3], F32)
            farr = geop.tile([B, 3], F32)
            for i in range(3):
                nc.vector.scalar_tensor_tensor(
                    out=farl[:, i:i + 1], in0=lg[i], scalar=1000.0,
                    in1=lc[i], op0=ALU.mult, op1=ALU.add,
                )
                nc.vector.scalar_tensor_tensor(
                    out=farr[:, i:i + 1], in0=rg[i], scalar=1000.0,
                    in1=rc[i], op0=ALU.mult, op1=ALU.add,
                )

            # ---------- tail assembly [B, 3, 11] ----------
            tail = geop.tile([B, 3, 11], F32)
            def _cp(k, out, in_):
                e = k % 3
                if e == 0:
                    nc.vector.tensor_copy(out=out, in_=in_)
                elif e == 1:
                    nc.scalar.copy(out=out, in_=in_)
                else:
                    nc.gpsimd.tensor_copy(out=out, in_=in_)

            _cp(0, tail[:, :, 0:1], rt[:, :, 68:69])   # l_c
            _cp(1, tail[:, :, 1:2], rt[:, :, 69:70])   # r_c
            _cp(2, tail[:, :, 2:3], rt[:, :, 70:71])   # face_centre
            _cp(0, tail[:, :, 3:4], gpl)
            _cp(1, tail[:, :, 4:5], gpr)
            _cp(2, tail[:, :, 5:6], gpm)
            _cp(0, tail[:, :, 6:7], farl)
            _cp(1, tail[:, :, 7:8], farr)
            _cp(2, tail[:, :, 8:9], gz[0:B, :])
            _cp(0, tail[:, :, 9:10], rg64)
            for i in range(3):
                _cp(i + 1, tail[:, i, 10:11], dist)

            # ---------- output DMAs ----------
            nc.sync.dma_start(out=out_p[:, :, 0:VSH], in_=rt[:, :, NG:NCOL])
            nc.scalar.dma_start(out=out_p[:, :, VSH:2 * VSH], in_=img)
            nc.gpsimd.dma_start(out=out_p[:, :, 2 * VSH:2 * VSH + 68], in_=rt[:, :, 0:68])
            nc.sync.dma_start(out=out_p[:, :, 2 * VSH + 68:OUTW], in_=tail)
    _legalize_waits(nc)
    return nc


def _prep(inputs):
    x = np.ascontiguousarray(inputs["x"].reshape(B, DIN), dtype=np.float32)
    enc_W = np.asarray(inputs["enc_W"], dtype=np.float32)
    basis = np.asarray(inputs["shape_basis"], dtype=np.float32)  # [400, V, 3]
    tmpl = np.asarray(inputs["v_template"], dtype=np.float32)    # [V, 3]
    cam = np.ascontiguousarray(
        np.asarray(inputs["camera_parameters"], dtype=np.float32).reshape(B, 12)
    )
    lm = np.asarray(inputs["landmarks"])
    mlm = np.asarray(inputs["masked_landmarks"])
    fmask = np.asarray(inputs["face_mask"])
    lmask = np.asarray(inputs["left_eyeball_mask"])
    rmask = np.asarray(inputs["right_eyeball_mask"])

    bias_row = np.concatenate([
        np.asarray(inputs["enc_b"], dtype=np.float32).reshape(1, LAT)[:, :400],
        np.asarray(inputs["enc_b"], dtype=np.float32).reshape(1, LAT)[:, 545:556],
        np.full((1, B), 1.0 / NCORES, np.float32),
        np.ones((1, B), np.float32),
    ], axis=1)

    # geometry columns [400, 72, 3] and [72, 3]
    fl_v = fmask[mlm]  # vert ids of the masked landmarks (in 0:3500)
    idx4 = lm[[19, 22, 25, 28]]
    idx2 = lm[[14, 18]]
    bG = np.empty((400, NG, 3), np.float32)
    bG[:, 0:68] = basis[:, fl_v, :]
    bG[:, 68] = basis[:, lmask, :].mean(axis=1)
    bG[:, 69] = basis[:, rmask, :].mean(axis=1)
    bG[:, 70] = basis[:, idx4, :].sum(axis=1) / 8.0 + basis[:, idx2, :].sum(axis=1) / 4.0
    bG[:, 71] = basis.mean(axis=1)
    tG = np.empty((NG, 3), np.float32)
    tG[0:68] = tmpl[fl_v]
    tG[68] = tmpl[lmask].mean(axis=0)
    tG[69] = tmpl[rmask].mean(axis=0)
    tG[70] = tmpl[idx4].sum(axis=0) / 8.0 + tmpl[idx2].sum(axis=0) / 4.0
    tG[71] = tmpl.mean(axis=0)

    # used encoder columns
    wcols = np.concatenate([enc_W[:, 0:400], enc_W[:, 545:556]], axis=1)  # [DIN, 411]

    in_maps = []
    for c in range(NCORES):
        k0 = c * KSH
        # x shard: [KSH, B] -> [21, 128, 7*64] (k = ci*896 + t*128 + p)
        xs = x[:, k0:k0 + KSH].T.reshape(NCHUNK, TPC, 128, B)
        xs = np.ascontiguousarray(xs.transpose(0, 2, 1, 3).reshape(NCHUNK, 128, TPC * B))
        ws = wcols[k0:k0 + KSH].reshape(NCHUNK, TPC, 128, NLAT)
        ws = np.ascontiguousarray(ws.transpose(0, 2, 1, 3).reshape(NCHUNK, 128, TPC * NLAT))
        # vert shard columns (pad past 3500 with zeros)
        v0 = c * VSH
        nv = max(0, min(VSH, VM - v0))
        bS = np.zeros((400, VSH, 3), np.float32)
        bS[:, :nv] = basis[:, v0:v0 + nv, :]
        tS = np.zeros((VSH, 3), np.float32)
        tS[:nv] = tmpl[v0:v0 + nv]
        bfull = np.concatenate([bG, bS], axis=1).transpose(0, 2, 1)  # [400, 3, 520]
        bpad = np.zeros((4 * 128, 3, NCOL), np.float32)
        bpad[0:400] = bfull
        bpad = bpad.reshape(4, 128, 3, NCOL)
        tfull = np.concatenate([tG, tS], axis=0).T.reshape(1, 3, NCOL)  # [1, 3, 520]
        in_maps.append({
            "x_sh": xs,
            "w_sh": ws,
            "bias_row": bias_row,
            "basis": bpad,
            "tmpl": np.ascontiguousarray(tfull),
            "cam": cam,
        })
    return in_maps


def _assemble(results):
    out = np.empty((B, NOUT, 3), np.float32)
    for c in range(NCORES):
        o = results[c]["out"]  # [B, 3, OUTW]
        v0 = c * VSH
        nv = max(0, min(VSH, VM - v0))
        if nv > 0:
            out[:, v0:v0 + nv, :] = o[:, :, 0:nv].transpose(0, 2, 1)
            out[:, VM + v0:VM + v0 + nv, :] = o[:, :, VSH:VSH + nv].transpose(0, 2, 1)
    o0 = results[0]["out"]
    out[:, 2 * VM:2 * VM + 68, :] = o0[:, :, 2 * VSH:2 * VSH + 68].transpose(0, 2, 1)
    out[:, 2 * VM + 68:NOUT, :] = o0[:, :, 2 * VSH + 68:OUTW].transpose(0, 2, 1)
    return out


def _run(inputs, trace=False):
    in_maps = _prep(inputs)
    nc = build_graph()
    res = run_bass_kernel_spmd(
        nc, in_maps, core_ids=list(range(NCORES)), trace=trace
    )
    out = _assemble(res.results)
    return np.ascontiguousarray(out), res


def kernel(**inputs):
    out, _ = _run(inputs, trace=False)
    return out
